# revision 11
# baseline (speedup 1.0000x reference)
"""MANN LSTM cell (scatter_memory) on 8 TRN2 NeuronCores.

Sharding: memory matrix m_tm1 and (M,B) addressing states sharded along the
memory axis (2048 rows/core); LSTM GEMM replicated on every core; one fused
AllGather collective carries the per-core partial read (m_k^T @ c_wr_k) and
per-core column-min of c_wu so every core can finish the write locally.

All matmuls run as float32r (bit-identical storage, single-pass PE) with the
moving dimension >= 256 so the PE runs at 1 cycle/row.  The cosine similarity
is computed transposed (keyT stationary, mT moving, N=512) and transposed back
per 128-row tile for the softmax-over-batch.

kernel(**inputs) takes FULL inputs, returns the FULL output tuple
(h, c, read, m, c_wu, c_wlu, c_wr) exactly like the reference.
"""
import numpy as np

import concourse.bacc as bacc
import concourse.mybir as mybir
from concourse import tile
from concourse.bass_utils import run_bass_kernel_spmd
from concourse.masks import make_identity

N_CORES = 8
B = 64
U = 512
IN_DIM = 512
M_FULL = 16384
MS = M_FULL // N_CORES       # 2048 memory rows per core
NT = MS // 128               # 16 row tiles per core
NG = 4                       # groups of 4 row tiles (512 rows) for cos
KT = IN_DIM // 128           # 4 contraction tiles
UT = U // 128                # 4 unit tiles
DECAY = 0.95
F32 = mybir.dt.float32
F32R = mybir.dt.float32r
EPS = 1e-12

_cache = {}


def _emit(nc, tc, h):
    gp = nc.gpsimd
    ve = nc.vector
    se = nc.scalar
    te = nc.tensor
    dma = nc.sync.dma_start
    X = mybir.AxisListType.X
    Alu = mybir.AluOpType
    Act = mybir.ActivationFunctionType

    def r(ap):
        return ap.bitcast(F32R)

    def f(ap):
        return ap.bitcast(F32)

    with (
        tc.tile_pool(name="const", bufs=1) as const,
        tc.tile_pool(name="big", bufs=1) as big,
        tc.tile_pool(name="sm1", bufs=1) as sm1,
        tc.tile_pool(name="sc2", bufs=2) as sc2,
        tc.tile_pool(name="mtg", bufs=1) as mtgp,
        tc.tile_pool(name="tr_ps", bufs=2, space="PSUM") as trp,
        tc.tile_pool(name="dram", bufs=1, space="DRAM") as dram,
    ):
        id128 = const.tile([128, 128], F32)
        make_identity(nc, id128[:])
        id128r = const.tile([128, 128], F32R)
        ve.tensor_copy(id128r[:], id128[:])

        # ---- big streaming loads issued first so they overlap the LSTM ----
        m_s = big.tile([128, NT, U], F32R)
        dma(m_s[:], r(h["m_in"].ap().rearrange("(t p) u -> p t u", p=128)))
        cwu_tm1 = big.tile([128, NT, B], F32)
        dma(cwu_tm1[:], h["cwu_in"].ap().rearrange("(t p) b -> p t b", p=128))
        cwlu_tm1 = big.tile([128, NT, B], F32)
        dma(cwlu_tm1[:], h["cwlu_in"].ap().rearrange("(t p) b -> p t b", p=128))
        cwr_tm1 = big.tile([128, NT, B], F32)
        dma(cwr_tm1[:], h["cwr_in"].ap().rearrange("(t p) b -> p t b", p=128))

        # ---- small LSTM state loads ----
        x_s = sm1.tile([B, IN_DIM], F32R)
        dma(x_s[:], r(h["x_in"].ap()))
        h_s = sm1.tile([B, U], F32R)
        dma(h_s[:], r(h["h_in"].ap()))
        c_s = sm1.tile([B, U], F32)
        dma(c_s[:], h["c_in"].ap())
        r_s = sm1.tile([B, U], F32R)
        dma(r_s[:], r(h["r_in"].ap()))
        wg_s = sm1.tile([1, 1], F32)
        dma(wg_s[:], h["wg"].ap())

        # write-gate scalars
        wg_sig = sm1.tile([1, 1], F32)
        se.activation(wg_sig[:], wg_s[:], Act.Sigmoid)
        onemwg = sm1.tile([1, 1], F32)
        ve.tensor_scalar(onemwg[:], wg_sig[:], -1.0, 1.0, Alu.mult, Alu.add)
        wg_b = sm1.tile([128, 1], F32)
        gp.partition_broadcast(wg_b[:], wg_sig[:])
        onemwg_b = sm1.tile([128, 1], F32)
        gp.partition_broadcast(onemwg_b[:], onemwg[:])

        bias_b = sm1.tile([B, 4 * U], F32)
        dma(bias_b[:], h["bias"].ap().broadcast_to((B, 4 * U)))

        # ---- LSTM GEMMs (replicated on every core) ----
        wk_view = h["wk"].ap().rearrange("(k p) j -> p k j", p=128)
        wr_view = h["wr"].ap().rearrange("(k p) j -> p k j", p=128)
        with (
            tc.tile_pool(name="wts", bufs=3) as wts,
            tc.tile_pool(name="g_ps", bufs=1, space="PSUM") as gpsp,
        ):
            # transpose x/h/r to contraction-major for the gate matmuls
            xT = sm1.tile([128, KT, B], F32R)
            hT = sm1.tile([128, KT, B], F32R)
            rT = sm1.tile([128, KT, B], F32R)
            for src, dstT in ((x_s, xT), (h_s, hT), (r_s, rT)):
                for k in range(KT):
                    tp = trp.tile([128, B], F32, name="tp")
                    te.transpose(r(tp[:]), src[:, k * 128:(k + 1) * 128],
                                 id128r[:B, :B])
                    ve.tensor_copy(dstT[:, k, :], tp[:])

            gates = []
            for j in range(4):
                wkc = wts.tile([128, KT, U], F32R, name="wc", tag="wc")
                dma(wkc[:], r(wk_view[:, :, j * U:(j + 1) * U]))
                wrc = wts.tile([128, KT, U], F32R, name="wc", tag="wc")
                dma(wrc[:], r(wr_view[:, :, j * U:(j + 1) * U]))
                gps = gpsp.tile([B, U], F32, name=f"g{j}", tag=f"g{j}")
                gates.append(gps)
                last_k = KT - 1
                for k in range(KT):
                    te.matmul(gps[:], xT[:, k, :], wkc[:, k, :],
                              start=(k == 0), stop=False)
                for k in range(KT):
                    is_last = (j != 0) and (k == last_k)
                    te.matmul(gps[:], hT[:, k, :], wrc[:, k, :],
                              start=False, stop=is_last)
                if j == 0:
                    wrr = wts.tile([128, KT, U], F32R, name="wc", tag="wc")
                    dma(wrr[:], r(wr_view[:, :, 4 * U:5 * U]))
                    for k in range(KT):
                        te.matmul(gps[:], rT[:, k, :], wrr[:, k, :],
                                  start=False, stop=(k == last_k))

            # gate nonlinearities
            def hard_sig(dst, ps, j):
                ve.tensor_tensor(dst[:], ps[:], bias_b[:, j * U:(j + 1) * U], Alu.add)
                ve.tensor_scalar(dst[:], dst[:], 0.2, 0.5, Alu.mult, Alu.add)
                ve.tensor_scalar(dst[:], dst[:], 0.0, 1.0, Alu.max, Alu.min)

            i_g = sm1.tile([B, U], F32)
            f_g = sm1.tile([B, U], F32)
            o_g = sm1.tile([B, U], F32)
            hard_sig(i_g, gates[0], 0)
            hard_sig(f_g, gates[1], 1)
            hard_sig(o_g, gates[3], 3)

            pre_c = sm1.tile([B, U], F32)
            ve.tensor_tensor(pre_c[:], gates[2][:], bias_b[:, 2 * U:3 * U], Alu.add)
            tanh_c = sm1.tile([B, U], F32)
            se.activation(tanh_c[:], pre_c[:], Act.Tanh)

        c_new = sm1.tile([B, U], F32)
        ve.tensor_tensor(c_new[:], f_g[:], c_s[:], Alu.mult)
        t2 = sc2.tile([B, U], F32, name="t2", tag="scr_k")
        ve.tensor_tensor(t2[:], i_g[:], tanh_c[:], Alu.mult)
        ve.tensor_tensor(c_new[:], c_new[:], t2[:], Alu.add)
        dma(h["c_out"].ap(), c_new[:])

        tanh_cn = sm1.tile([B, U], F32)
        se.activation(tanh_cn[:], c_new[:], Act.Tanh)
        h_new = sm1.tile([B, U], F32R)
        ve.tensor_tensor(h_new[:], o_g[:], tanh_cn[:], Alu.mult)
        dma(h["h_out"].ap(), f(h_new[:]))

        # ---- key normalization: n_key = h / max(||h||, eps) ----
        scr_k = sc2.tile([B, U], F32, name="scr_k")
        ksum = sm1.tile([B, 1], F32)
        se.activation(scr_k[:], f(h_new[:]), Act.Square, accum_out=ksum[:])
        ve.tensor_scalar_max(ksum[:], ksum[:], EPS)
        ksq = sm1.tile([B, 1], F32)
        se.activation(ksq[:], ksum[:], Act.Sqrt)
        rkn = sm1.tile([B, 1], F32)
        ve.reciprocal(rkn[:], ksq[:])
        nkey = sm1.tile([B, U], F32R)
        ve.tensor_scalar_mul(nkey[:], f(h_new[:]), rkn[:])

        keyT = sm1.tile([128, UT, B], F32R)
        for u in range(UT):
            tp = trp.tile([128, B], F32, name="tp")
            te.transpose(r(tp[:]), nkey[:, u * 128:(u + 1) * 128], id128r[:B, :B])
            ve.tensor_copy(keyT[:, u, :], tp[:])

        # ---- c_ww = wg*c_wr_tm1 + (1-wg) + c_wlu_tm1 (full width) ----
        cww_all = big.tile([128, NT, B], F32)
        ve.tensor_scalar(cww_all[:], cwr_tm1[:], wg_b[:], onemwg_b[:], Alu.mult, Alu.add)
        ve.tensor_tensor(cww_all[:], cww_all[:], cwlu_tm1[:], Alu.add)

        cwr_all = big.tile([128, NT, B], F32R)
        cwu_all = big.tile([128, NT, B], F32)
        cwlu_all = big.tile([128, NT, B], F32)
        cos_all = big.tile([128, NT, B], F32)

        # ---- 1/||m_row|| for every local row ----
        rmn_all = sm1.tile([128, NT], F32)
        for t in range(NT):
            scr2 = sc2.tile([128, U], F32, name="scr2")
            se.activation(scr2[:], f(m_s[:, t, :]), Act.Square,
                          accum_out=rmn_all[:, t:t + 1])
        ve.tensor_scalar_max(rmn_all[:], rmn_all[:], EPS)
        msq_all = sm1.tile([128, NT], F32)
        se.activation(msq_all[:], rmn_all[:], Act.Sqrt)
        ve.reciprocal(rmn_all[:], msq_all[:])

        with (
            tc.tile_pool(name="cos_ps", bufs=2, space="PSUM") as cosp,
            tc.tile_pool(name="rd_ps", bufs=1, space="PSUM") as rdp,
            tc.tile_pool(name="wr_ps", bufs=2, space="PSUM") as wrp,
        ):
            read_ps = rdp.tile([B, U], F32)

            # ---- cos^T = n_key^T-contraction matmuls, 512-row groups ----
            for g in range(NG):
                mtg = mtgp.tile([128, UT, 512], F32R, name="mtg")
                for ti in range(4):
                    t = g * 4 + ti
                    for u in range(UT):
                        tp = trp.tile([128, 128], F32, name="tp")
                        te.transpose(r(tp[:]), m_s[:, t, u * 128:(u + 1) * 128],
                                     id128r[:])
                        ve.tensor_copy(mtg[:, u, ti * 128:(ti + 1) * 128], tp[:])
                cosT_ps = cosp.tile([B, 512], F32, name="cosT_ps")
                for u in range(UT):
                    te.matmul(cosT_ps[:], keyT[:, u, :], mtg[:, u, :],
                              start=(u == 0), stop=(u == UT - 1))
                cosT_sb = sc2.tile([B, 512], F32, name="cosT_sb")
                ve.tensor_copy(cosT_sb[:], cosT_ps[:])
                for ti in range(4):
                    t = g * 4 + ti
                    tpb = trp.tile([128, B], F32, name="tp")
                    te.transpose(tpb[:], cosT_sb[:, ti * 128:(ti + 1) * 128],
                                 id128[:B, :B])
                    ve.tensor_copy(cos_all[:, t, :], tpb[:])

            # ---- softmax over batch (full width, row-broadcast APs) ----
            def bcast(ap2d):  # (128, NT) -> (128, NT, B) step-0 broadcast
                return ap2d.unsqueeze(2).broadcast_to((128, NT, B))

            ve.tensor_tensor(cos_all[:], cos_all[:], bcast(rmn_all[:]), Alu.mult)
            se.activation(cos_all[:], cos_all[:], Act.Exp)
            rsum = sm1.tile([128, NT], F32)
            ve.tensor_reduce(rsum[:], cos_all[:], X, Alu.add)
            rrec = sm1.tile([128, NT], F32)
            ve.reciprocal(rrec[:], rsum[:])
            ve.tensor_tensor(cwr_all[:], cos_all[:], bcast(rrec[:]), Alu.mult)

            # ---- partial read, accumulated over local row tiles ----
            for t in range(NT):
                te.matmul(read_ps[:], cwr_all[:, t, :], m_s[:, t, :],
                          start=(t == 0), stop=(t == NT - 1))

            # ---- write term s = m + c_ww @ h (independent of the collective) ----
            s_all = big.tile([128, NT, U], F32)
            for t in range(NT):
                tpw = trp.tile([B, 128], F32, name="tp")
                te.transpose(tpw[:], cww_all[:, t, :], id128[:])
                cwwT = sc2.tile([B, 128], F32R, name="cwwT")
                ve.tensor_copy(cwwT[:], tpw[:])
                wr_ps = wrp.tile([128, U], F32, name="wr_ps")
                te.matmul(wr_ps[:], cwwT[:], h_new[:], start=True, stop=True)
                ve.tensor_tensor(s_all[:, t, :], f(m_s[:, t, :]), wr_ps[:], Alu.add)

            # ---- usage update (full width); cwlu_all doubles as scratch ----
            ve.tensor_tensor(cwlu_all[:], f(cwr_all[:]), cww_all[:], Alu.add)
            ve.tensor_scalar(cwu_all[:], cwu_tm1[:], DECAY, None, Alu.mult)
            ve.tensor_tensor(cwu_all[:], cwu_all[:], cwlu_all[:], Alu.add)

            # local column-min over the 2048 local rows
            minacc = sm1.tile([128, B], F32)
            ve.tensor_reduce(minacc[:], cwu_all[:].rearrange("p t b -> p b t"), X,
                             Alu.min)
            tpm = trp.tile([B, 128], F32, name="tp")
            te.transpose(tpm[:], minacc[:], id128[:])
            mint = sm1.tile([B, 128], F32)
            ve.tensor_copy(mint[:], tpm[:])
            colmin = sm1.tile([B, 1], F32)
            ve.tensor_reduce(colmin[:], mint[:], X, Alu.min)

            # ---- one fused collective: [read_part | colmin] AllGather ----
            cc_in = sm1.tile([B, U + 1], F32)
            ve.tensor_copy(cc_in[:, 0:U], read_ps[:])
            ve.tensor_copy(cc_in[:, U:U + 1], colmin[:])
            ccb_in = dram.tile([B, U + 1], F32)
            ccb_out = dram.tile([N_CORES * B, U + 1], F32, addr_space="Shared")
            dma(ccb_in[:], cc_in[:])
            gp.collective_compute(
                "AllGather",
                Alu.bypass,
                ins=[ccb_in[:].opt()],
                outs=[ccb_out[:].opt()],
                replica_groups=[list(range(N_CORES))],
            )
            gath = sm1.tile([B, N_CORES, U + 1], F32)
            dma(gath[:], ccb_out[:].rearrange("(c b) f -> b c f", b=B))

            read_full = sm1.tile([B, U], F32)
            ve.tensor_tensor(read_full[:], gath[:, 0, 0:U], gath[:, 1, 0:U], Alu.add)
            for c in range(2, N_CORES):
                ve.tensor_tensor(read_full[:], read_full[:], gath[:, c, 0:U], Alu.add)
            dma(h["read_out"].ap(), read_full[:])

            gmin = sm1.tile([B, 1], F32)
            ve.tensor_tensor(gmin[:], gath[:, 0, U:U + 1], gath[:, 1, U:U + 1],
                             Alu.min)
            for c in range(2, N_CORES):
                ve.tensor_tensor(gmin[:], gmin[:], gath[:, c, U:U + 1], Alu.min)

            tpg = trp.tile([1, B], F32, name="tp")
            te.transpose(tpg[:], gmin[:], id128[:B, :B])
            gminrow = sm1.tile([1, B], F32)
            ve.tensor_copy(gminrow[:], tpg[:])
            gmin_b = sm1.tile([128, B], F32)
            gp.partition_broadcast(gmin_b[:], gminrow[:])

            # ---- least-used mask (full width) ----
            gmin_bc = gmin_b[:].unsqueeze(1).broadcast_to((128, NT, B))
            ve.tensor_tensor(cwlu_all[:], cwu_all[:], gmin_bc, Alu.is_le)
            rowflag = sm1.tile([128, NT], F32)
            ve.tensor_reduce(rowflag[:], cwlu_all[:], X, Alu.max)

            # ---- memory write: m_new = s - m*rowflag ----
            m_out_view = h["m_out"].ap().rearrange("(t p) u -> p t u", p=128)
            for t in range(NT):
                corr = sc2.tile([128, U], F32, name="scr2", tag="scr2")
                se.activation(corr[:], f(m_s[:, t, :]), Act.Copy,
                              scale=rowflag[:, t:t + 1])
                ve.tensor_tensor(s_all[:, t, :], s_all[:, t, :], corr[:],
                                 Alu.subtract)
                dma(m_out_view[:, t, :], s_all[:, t, :])

        # ---- remaining outputs ----
        dma(h["cwr_out"].ap().rearrange("(t p) b -> p t b", p=128), f(cwr_all[:]))
        dma(h["cwu_out"].ap().rearrange("(t p) b -> p t b", p=128), cwu_all[:])
        dma(h["cwlu_out"].ap().rearrange("(t p) b -> p t b", p=128), cwlu_all[:])


def _build():
    nc = bacc.Bacc("TRN2", target_bir_lowering=False, debug=False,
                   num_devices=N_CORES)
    h = {}
    h["x_in"] = nc.dram_tensor("x_in", [B, IN_DIM], F32, kind="ExternalInput")
    h["h_in"] = nc.dram_tensor("h_in", [B, U], F32, kind="ExternalInput")
    h["c_in"] = nc.dram_tensor("c_in", [B, U], F32, kind="ExternalInput")
    h["r_in"] = nc.dram_tensor("r_in", [B, U], F32, kind="ExternalInput")
    h["m_in"] = nc.dram_tensor("m_in", [MS, U], F32, kind="ExternalInput")
    h["cwu_in"] = nc.dram_tensor("cwu_in", [MS, B], F32, kind="ExternalInput")
    h["cwlu_in"] = nc.dram_tensor("cwlu_in", [MS, B], F32, kind="ExternalInput")
    h["cwr_in"] = nc.dram_tensor("cwr_in", [MS, B], F32, kind="ExternalInput")
    h["wk"] = nc.dram_tensor("wk", [IN_DIM, 4 * U], F32, kind="ExternalInput")
    h["wr"] = nc.dram_tensor("wr", [U, 5 * U], F32, kind="ExternalInput")
    h["bias"] = nc.dram_tensor("bias", [1, 4 * U], F32, kind="ExternalInput")
    h["wg"] = nc.dram_tensor("wg", [1, 1], F32, kind="ExternalInput")

    h["h_out"] = nc.dram_tensor("h_out", [B, U], F32, kind="ExternalOutput")
    h["c_out"] = nc.dram_tensor("c_out", [B, U], F32, kind="ExternalOutput")
    h["read_out"] = nc.dram_tensor("read_out", [B, U], F32, kind="ExternalOutput")
    h["m_out"] = nc.dram_tensor("m_out", [MS, U], F32, kind="ExternalOutput")
    h["cwu_out"] = nc.dram_tensor("cwu_out", [MS, B], F32, kind="ExternalOutput")
    h["cwlu_out"] = nc.dram_tensor("cwlu_out", [MS, B], F32, kind="ExternalOutput")
    h["cwr_out"] = nc.dram_tensor("cwr_out", [MS, B], F32, kind="ExternalOutput")

    with tile.TileContext(nc) as tc:
        _emit(nc, tc, h)
    nc.compile()
    return nc


def _get_nc():
    if "nc" not in _cache:
        _cache["nc"] = _build()
    return _cache["nc"]


def _run(inputs, trace=False):
    nc = _get_nc()
    f32 = np.float32

    def arr(x):
        return np.ascontiguousarray(np.asarray(x, dtype=f32))

    full = {k: arr(v) for k, v in inputs.items()}
    in_maps = []
    for k in range(N_CORES):
        sl = slice(k * MS, (k + 1) * MS)
        in_maps.append({
            "x_in": full["inputs"],
            "h_in": full["h_tm1"],
            "c_in": full["c_tm1"],
            "r_in": full["r_tm1"],
            "m_in": full["m_tm1"][sl],
            "cwu_in": full["c_wu_tm1"][sl],
            "cwlu_in": full["c_wlu_tm1"][sl],
            "cwr_in": full["c_wr_tm1"][sl],
            "wk": full["kernel"],
            "wr": full["recurrent_kernel"],
            "bias": full["bias"].reshape(1, 4 * U),
            "wg": full["write_gate"].reshape(1, 1),
        })
    br = run_bass_kernel_spmd(nc, in_maps, core_ids=list(range(N_CORES)),
                              trace=trace)
    res = br.results
    h_out = res[0]["h_out"]
    c_out = res[0]["c_out"]
    read_out = res[0]["read_out"]
    m_out = np.concatenate([res[k]["m_out"] for k in range(N_CORES)], axis=0)
    cwu_out = np.concatenate([res[k]["cwu_out"] for k in range(N_CORES)], axis=0)
    cwlu_out = np.concatenate([res[k]["cwlu_out"] for k in range(N_CORES)], axis=0)
    cwr_out = np.concatenate([res[k]["cwr_out"] for k in range(N_CORES)], axis=0)
    out = (h_out, c_out, read_out, m_out, cwu_out, cwlu_out, cwr_out)
    return out, br


def kernel(**inputs):
    out, _ = _run(inputs, trace=False)
    return out


# revision 12
# speedup vs baseline: 1.0020x; 1.0020x over previous
"""MANN LSTM cell (scatter_memory) on 8 TRN2 NeuronCores.

Sharding: memory matrix m_tm1 and (M,B) addressing states sharded along the
memory axis (2048 rows/core); LSTM GEMM replicated on every core; one fused
AllGather collective carries the per-core partial read (m_k^T @ c_wr_k) and
per-core column-min of c_wu so every core can finish the write locally.

All matmuls run as float32r (bit-identical storage, single-pass PE) with the
moving dimension >= 256 so the PE runs at 1 cycle/row.  The cosine similarity
is computed transposed (keyT stationary, mT moving, N=512) and transposed back
per 128-row tile for the softmax-over-batch.

kernel(**inputs) takes FULL inputs, returns the FULL output tuple
(h, c, read, m, c_wu, c_wlu, c_wr) exactly like the reference.
"""
import numpy as np

import concourse.bacc as bacc
import concourse.mybir as mybir
from concourse import tile
from concourse.bass_utils import run_bass_kernel_spmd
from concourse.masks import make_identity

N_CORES = 8
B = 64
U = 512
IN_DIM = 512
M_FULL = 16384
MS = M_FULL // N_CORES       # 2048 memory rows per core
NT = MS // 128               # 16 row tiles per core
NG = 4                       # groups of 4 row tiles (512 rows) for cos
KT = IN_DIM // 128           # 4 contraction tiles
UT = U // 128                # 4 unit tiles
DECAY = 0.95
F32 = mybir.dt.float32
F32R = mybir.dt.float32r
EPS = 1e-12

_cache = {}


def _emit(nc, tc, h):
    gp = nc.gpsimd
    ve = nc.vector
    se = nc.scalar
    te = nc.tensor
    dma = nc.sync.dma_start
    dma2 = nc.scalar.dma_start
    X = mybir.AxisListType.X
    Alu = mybir.AluOpType
    Act = mybir.ActivationFunctionType

    def r(ap):
        return ap.bitcast(F32R)

    def f(ap):
        return ap.bitcast(F32)

    with (
        tc.tile_pool(name="const", bufs=1) as const,
        tc.tile_pool(name="big", bufs=1) as big,
        tc.tile_pool(name="sm1", bufs=1) as sm1,
        tc.tile_pool(name="sc2", bufs=2) as sc2,
        tc.tile_pool(name="mtg", bufs=1) as mtgp,
        tc.tile_pool(name="tr_ps", bufs=2, space="PSUM") as trp,
        tc.tile_pool(name="dram", bufs=1, space="DRAM") as dram,
    ):
        id128 = const.tile([128, 128], F32)
        make_identity(nc, id128[:])
        id128r = const.tile([128, 128], F32R)
        ve.tensor_copy(id128r[:], id128[:])

        # ---- big streaming loads issued first so they overlap the LSTM ----
        m_s = big.tile([128, NT, U], F32R)
        m_view = r(h["m_in"].ap().rearrange("(t p) u -> p t u", p=128))
        dma(m_s[:, 0:NT // 2, :], m_view[:, 0:NT // 2, :])
        dma2(m_s[:, NT // 2:NT, :], m_view[:, NT // 2:NT, :])
        cwu_tm1 = big.tile([128, NT, B], F32)
        dma2(cwu_tm1[:], h["cwu_in"].ap().rearrange("(t p) b -> p t b", p=128))
        cwlu_tm1 = big.tile([128, NT, B], F32)
        dma2(cwlu_tm1[:], h["cwlu_in"].ap().rearrange("(t p) b -> p t b", p=128))
        cwr_tm1 = big.tile([128, NT, B], F32)
        dma2(cwr_tm1[:], h["cwr_in"].ap().rearrange("(t p) b -> p t b", p=128))

        # ---- small LSTM state loads ----
        x_s = sm1.tile([B, IN_DIM], F32R)
        dma(x_s[:], r(h["x_in"].ap()))
        h_s = sm1.tile([B, U], F32R)
        dma(h_s[:], r(h["h_in"].ap()))
        c_s = sm1.tile([B, U], F32)
        dma(c_s[:], h["c_in"].ap())
        r_s = sm1.tile([B, U], F32R)
        dma(r_s[:], r(h["r_in"].ap()))
        wg_s = sm1.tile([1, 1], F32)
        dma(wg_s[:], h["wg"].ap())

        # write-gate scalars
        wg_sig = sm1.tile([1, 1], F32)
        se.activation(wg_sig[:], wg_s[:], Act.Sigmoid)
        onemwg = sm1.tile([1, 1], F32)
        ve.tensor_scalar(onemwg[:], wg_sig[:], -1.0, 1.0, Alu.mult, Alu.add)
        wg_b = sm1.tile([128, 1], F32)
        gp.partition_broadcast(wg_b[:], wg_sig[:])
        onemwg_b = sm1.tile([128, 1], F32)
        gp.partition_broadcast(onemwg_b[:], onemwg[:])

        bias_b = sm1.tile([B, 4 * U], F32)
        dma(bias_b[:], h["bias"].ap().broadcast_to((B, 4 * U)))

        # ---- LSTM GEMMs (replicated on every core) ----
        wk_view = h["wk"].ap().rearrange("(k p) j -> p k j", p=128)
        wr_view = h["wr"].ap().rearrange("(k p) j -> p k j", p=128)
        with (
            tc.tile_pool(name="wts", bufs=3) as wts,
            tc.tile_pool(name="g_ps", bufs=1, space="PSUM") as gpsp,
        ):
            # transpose x/h/r to contraction-major for the gate matmuls
            xT = sm1.tile([128, KT, B], F32R)
            hT = sm1.tile([128, KT, B], F32R)
            rT = sm1.tile([128, KT, B], F32R)
            for src, dstT in ((x_s, xT), (h_s, hT), (r_s, rT)):
                for k in range(KT):
                    tp = trp.tile([128, B], F32, name="tp")
                    te.transpose(r(tp[:]), src[:, k * 128:(k + 1) * 128],
                                 id128r[:B, :B])
                    ve.tensor_copy(dstT[:, k, :], tp[:])

            gates = []
            for j in range(4):
                wkc = wts.tile([128, KT, U], F32R, name="wc", tag="wc")
                dma(wkc[:], r(wk_view[:, :, j * U:(j + 1) * U]))
                wrc = wts.tile([128, KT, U], F32R, name="wc", tag="wc")
                dma(wrc[:], r(wr_view[:, :, j * U:(j + 1) * U]))
                gps = gpsp.tile([B, U], F32, name=f"g{j}", tag=f"g{j}")
                gates.append(gps)
                last_k = KT - 1
                for k in range(KT):
                    te.matmul(gps[:], xT[:, k, :], wkc[:, k, :],
                              start=(k == 0), stop=False)
                for k in range(KT):
                    is_last = (j != 0) and (k == last_k)
                    te.matmul(gps[:], hT[:, k, :], wrc[:, k, :],
                              start=False, stop=is_last)
                if j == 0:
                    wrr = wts.tile([128, KT, U], F32R, name="wc", tag="wc")
                    dma(wrr[:], r(wr_view[:, :, 4 * U:5 * U]))
                    for k in range(KT):
                        te.matmul(gps[:], rT[:, k, :], wrr[:, k, :],
                                  start=False, stop=(k == last_k))

            # gate nonlinearities
            def hard_sig(dst, ps, j):
                ve.tensor_tensor(dst[:], ps[:], bias_b[:, j * U:(j + 1) * U], Alu.add)
                ve.tensor_scalar(dst[:], dst[:], 0.2, 0.5, Alu.mult, Alu.add)
                ve.tensor_scalar(dst[:], dst[:], 0.0, 1.0, Alu.max, Alu.min)

            i_g = sm1.tile([B, U], F32)
            f_g = sm1.tile([B, U], F32)
            o_g = sm1.tile([B, U], F32)
            hard_sig(i_g, gates[0], 0)
            hard_sig(f_g, gates[1], 1)
            hard_sig(o_g, gates[3], 3)

            pre_c = sm1.tile([B, U], F32)
            ve.tensor_tensor(pre_c[:], gates[2][:], bias_b[:, 2 * U:3 * U], Alu.add)
            tanh_c = sm1.tile([B, U], F32)
            se.activation(tanh_c[:], pre_c[:], Act.Tanh)

        c_new = sm1.tile([B, U], F32)
        ve.tensor_tensor(c_new[:], f_g[:], c_s[:], Alu.mult)
        t2 = sc2.tile([B, U], F32, name="t2", tag="scr_k")
        ve.tensor_tensor(t2[:], i_g[:], tanh_c[:], Alu.mult)
        ve.tensor_tensor(c_new[:], c_new[:], t2[:], Alu.add)
        dma(h["c_out"].ap(), c_new[:])

        tanh_cn = sm1.tile([B, U], F32)
        se.activation(tanh_cn[:], c_new[:], Act.Tanh)
        h_new = sm1.tile([B, U], F32R)
        ve.tensor_tensor(h_new[:], o_g[:], tanh_cn[:], Alu.mult)
        dma(h["h_out"].ap(), f(h_new[:]))

        # ---- key normalization: n_key = h / max(||h||, eps) ----
        scr_k = sc2.tile([B, U], F32, name="scr_k")
        ksum = sm1.tile([B, 1], F32)
        se.activation(scr_k[:], f(h_new[:]), Act.Square, accum_out=ksum[:])
        ve.tensor_scalar_max(ksum[:], ksum[:], EPS)
        ksq = sm1.tile([B, 1], F32)
        se.activation(ksq[:], ksum[:], Act.Sqrt)
        rkn = sm1.tile([B, 1], F32)
        ve.reciprocal(rkn[:], ksq[:])
        nkey = sm1.tile([B, U], F32R)
        ve.tensor_scalar_mul(nkey[:], f(h_new[:]), rkn[:])

        keyT = sm1.tile([128, UT, B], F32R)
        for u in range(UT):
            tp = trp.tile([128, B], F32, name="tp")
            te.transpose(r(tp[:]), nkey[:, u * 128:(u + 1) * 128], id128r[:B, :B])
            ve.tensor_copy(keyT[:, u, :], tp[:])

        # ---- c_ww = wg*c_wr_tm1 + (1-wg) + c_wlu_tm1 (full width) ----
        cww_all = big.tile([128, NT, B], F32)
        ve.tensor_scalar(cww_all[:], cwr_tm1[:], wg_b[:], onemwg_b[:], Alu.mult, Alu.add)
        ve.tensor_tensor(cww_all[:], cww_all[:], cwlu_tm1[:], Alu.add)

        cwr_all = big.tile([128, NT, B], F32R)
        cwu_all = big.tile([128, NT, B], F32)
        cwlu_all = big.tile([128, NT, B], F32)
        cos_all = big.tile([128, NT, B], F32)

        # ---- 1/||m_row|| for every local row ----
        rmn_all = sm1.tile([128, NT], F32)
        for t in range(NT):
            scr2 = sc2.tile([128, U], F32, name="scr2")
            se.activation(scr2[:], f(m_s[:, t, :]), Act.Square,
                          accum_out=rmn_all[:, t:t + 1])
        ve.tensor_scalar_max(rmn_all[:], rmn_all[:], EPS)
        msq_all = sm1.tile([128, NT], F32)
        se.activation(msq_all[:], rmn_all[:], Act.Sqrt)
        ve.reciprocal(rmn_all[:], msq_all[:])

        with (
            tc.tile_pool(name="cos_ps", bufs=2, space="PSUM") as cosp,
            tc.tile_pool(name="rd_ps", bufs=1, space="PSUM") as rdp,
            tc.tile_pool(name="wr_ps", bufs=2, space="PSUM") as wrp,
        ):
            read_ps = rdp.tile([B, U], F32)

            # ---- cos^T = n_key^T-contraction matmuls, 512-row groups ----
            for g in range(NG):
                mtg = mtgp.tile([128, UT, 512], F32R, name="mtg")
                for ti in range(4):
                    t = g * 4 + ti
                    for u in range(UT):
                        tp = trp.tile([128, 128], F32, name="tp")
                        te.transpose(r(tp[:]), m_s[:, t, u * 128:(u + 1) * 128],
                                     id128r[:])
                        ve.tensor_copy(mtg[:, u, ti * 128:(ti + 1) * 128], tp[:])
                cosT_ps = cosp.tile([B, 512], F32, name="cosT_ps")
                for u in range(UT):
                    te.matmul(cosT_ps[:], keyT[:, u, :], mtg[:, u, :],
                              start=(u == 0), stop=(u == UT - 1))
                cosT_sb = sc2.tile([B, 512], F32, name="cosT_sb")
                ve.tensor_copy(cosT_sb[:], cosT_ps[:])
                for ti in range(4):
                    t = g * 4 + ti
                    tpb = trp.tile([128, B], F32, name="tp")
                    te.transpose(tpb[:], cosT_sb[:, ti * 128:(ti + 1) * 128],
                                 id128[:B, :B])
                    ve.tensor_copy(cos_all[:, t, :], tpb[:])

            # ---- softmax over batch (full width, row-broadcast APs) ----
            def bcast(ap2d):  # (128, NT) -> (128, NT, B) step-0 broadcast
                return ap2d.unsqueeze(2).broadcast_to((128, NT, B))

            ve.tensor_tensor(cos_all[:], cos_all[:], bcast(rmn_all[:]), Alu.mult)
            se.activation(cos_all[:], cos_all[:], Act.Exp)
            rsum = sm1.tile([128, NT], F32)
            ve.tensor_reduce(rsum[:], cos_all[:], X, Alu.add)
            rrec = sm1.tile([128, NT], F32)
            ve.reciprocal(rrec[:], rsum[:])
            ve.tensor_tensor(cwr_all[:], cos_all[:], bcast(rrec[:]), Alu.mult)

            # ---- partial read, accumulated over local row tiles ----
            for t in range(NT):
                te.matmul(read_ps[:], cwr_all[:, t, :], m_s[:, t, :],
                          start=(t == 0), stop=(t == NT - 1))

            # ---- write term s = m + c_ww @ h (independent of the collective) ----
            s_all = big.tile([128, NT, U], F32)
            for t in range(NT):
                tpw = trp.tile([B, 128], F32, name="tp")
                te.transpose(tpw[:], cww_all[:, t, :], id128[:])
                cwwT = sc2.tile([B, 128], F32R, name="cwwT")
                ve.tensor_copy(cwwT[:], tpw[:])
                wr_ps = wrp.tile([128, U], F32, name="wr_ps")
                te.matmul(wr_ps[:], cwwT[:], h_new[:], start=True, stop=True)
                ve.tensor_tensor(s_all[:, t, :], f(m_s[:, t, :]), wr_ps[:], Alu.add)

            # ---- usage update (full width); cwlu_all doubles as scratch ----
            ve.tensor_tensor(cwlu_all[:], f(cwr_all[:]), cww_all[:], Alu.add)
            ve.tensor_scalar(cwu_all[:], cwu_tm1[:], DECAY, None, Alu.mult)
            ve.tensor_tensor(cwu_all[:], cwu_all[:], cwlu_all[:], Alu.add)

            # local column-min over the 2048 local rows
            minacc = sm1.tile([128, B], F32)
            ve.tensor_reduce(minacc[:], cwu_all[:].rearrange("p t b -> p b t"), X,
                             Alu.min)
            tpm = trp.tile([B, 128], F32, name="tp")
            te.transpose(tpm[:], minacc[:], id128[:])
            mint = sm1.tile([B, 128], F32)
            ve.tensor_copy(mint[:], tpm[:])
            colmin = sm1.tile([B, 1], F32)
            ve.tensor_reduce(colmin[:], mint[:], X, Alu.min)

            # ---- collective 1: tiny column-min AllGather (gates the tail) ----
            ccm_in = dram.tile([B, 1], F32)
            ccm_out = dram.tile([N_CORES * B, 1], F32, addr_space="Shared")
            dma(ccm_in[:], colmin[:])
            gp.collective_compute(
                "AllGather",
                Alu.bypass,
                ins=[ccm_in[:].opt()],
                outs=[ccm_out[:].opt()],
                replica_groups=[list(range(N_CORES))],
            )
            gathm = sm1.tile([B, N_CORES], F32)
            dma(gathm[:], ccm_out[:].rearrange("(c b) f -> b (c f)", b=B))
            gmin = sm1.tile([B, 1], F32)
            ve.tensor_reduce(gmin[:], gathm[:], X, Alu.min)

            # ---- collective 2: read partials (off the critical path) ----
            cc_in = sm1.tile([B, U], F32)
            ve.tensor_copy(cc_in[:], read_ps[:])
            ccb_in = dram.tile([B, U], F32)
            ccb_out = dram.tile([N_CORES * B, U], F32, addr_space="Shared")
            dma(ccb_in[:], cc_in[:])
            gp.collective_compute(
                "AllGather",
                Alu.bypass,
                ins=[ccb_in[:].opt()],
                outs=[ccb_out[:].opt()],
                replica_groups=[list(range(N_CORES))],
            )
            gath = sm1.tile([B, N_CORES, U], F32)
            dma2(gath[:], ccb_out[:].rearrange("(c b) f -> b c f", b=B))

            read_full = sm1.tile([B, U], F32)
            ve.tensor_tensor(read_full[:], gath[:, 0, :], gath[:, 1, :], Alu.add)
            for c in range(2, N_CORES):
                ve.tensor_tensor(read_full[:], read_full[:], gath[:, c, :], Alu.add)
            dma2(h["read_out"].ap(), read_full[:])

            tpg = trp.tile([1, B], F32, name="tp")
            te.transpose(tpg[:], gmin[:], id128[:B, :B])
            gminrow = sm1.tile([1, B], F32)
            ve.tensor_copy(gminrow[:], tpg[:])
            gmin_b = sm1.tile([128, B], F32)
            gp.partition_broadcast(gmin_b[:], gminrow[:])

            # ---- least-used mask (full width) ----
            gmin_bc = gmin_b[:].unsqueeze(1).broadcast_to((128, NT, B))
            ve.tensor_tensor(cwlu_all[:], cwu_all[:], gmin_bc, Alu.is_le)
            rowflag = sm1.tile([128, NT], F32)
            ve.tensor_reduce(rowflag[:], cwlu_all[:], X, Alu.max)

            # ---- memory write: m_new = s - m*rowflag ----
            m_out_view = h["m_out"].ap().rearrange("(t p) u -> p t u", p=128)
            for t in range(NT):
                corr = sc2.tile([128, U], F32, name="scr2", tag="scr2")
                se.activation(corr[:], f(m_s[:, t, :]), Act.Copy,
                              scale=rowflag[:, t:t + 1])
                ve.tensor_tensor(s_all[:, t, :], s_all[:, t, :], corr[:],
                                 Alu.subtract)
                (dma if t % 2 == 0 else dma2)(m_out_view[:, t, :], s_all[:, t, :])

        # ---- remaining outputs ----
        dma(h["cwr_out"].ap().rearrange("(t p) b -> p t b", p=128), f(cwr_all[:]))
        dma(h["cwu_out"].ap().rearrange("(t p) b -> p t b", p=128), cwu_all[:])
        dma(h["cwlu_out"].ap().rearrange("(t p) b -> p t b", p=128), cwlu_all[:])


def _build():
    nc = bacc.Bacc("TRN2", target_bir_lowering=False, debug=False,
                   num_devices=N_CORES)
    h = {}
    h["x_in"] = nc.dram_tensor("x_in", [B, IN_DIM], F32, kind="ExternalInput")
    h["h_in"] = nc.dram_tensor("h_in", [B, U], F32, kind="ExternalInput")
    h["c_in"] = nc.dram_tensor("c_in", [B, U], F32, kind="ExternalInput")
    h["r_in"] = nc.dram_tensor("r_in", [B, U], F32, kind="ExternalInput")
    h["m_in"] = nc.dram_tensor("m_in", [MS, U], F32, kind="ExternalInput")
    h["cwu_in"] = nc.dram_tensor("cwu_in", [MS, B], F32, kind="ExternalInput")
    h["cwlu_in"] = nc.dram_tensor("cwlu_in", [MS, B], F32, kind="ExternalInput")
    h["cwr_in"] = nc.dram_tensor("cwr_in", [MS, B], F32, kind="ExternalInput")
    h["wk"] = nc.dram_tensor("wk", [IN_DIM, 4 * U], F32, kind="ExternalInput")
    h["wr"] = nc.dram_tensor("wr", [U, 5 * U], F32, kind="ExternalInput")
    h["bias"] = nc.dram_tensor("bias", [1, 4 * U], F32, kind="ExternalInput")
    h["wg"] = nc.dram_tensor("wg", [1, 1], F32, kind="ExternalInput")

    h["h_out"] = nc.dram_tensor("h_out", [B, U], F32, kind="ExternalOutput")
    h["c_out"] = nc.dram_tensor("c_out", [B, U], F32, kind="ExternalOutput")
    h["read_out"] = nc.dram_tensor("read_out", [B, U], F32, kind="ExternalOutput")
    h["m_out"] = nc.dram_tensor("m_out", [MS, U], F32, kind="ExternalOutput")
    h["cwu_out"] = nc.dram_tensor("cwu_out", [MS, B], F32, kind="ExternalOutput")
    h["cwlu_out"] = nc.dram_tensor("cwlu_out", [MS, B], F32, kind="ExternalOutput")
    h["cwr_out"] = nc.dram_tensor("cwr_out", [MS, B], F32, kind="ExternalOutput")

    with tile.TileContext(nc) as tc:
        _emit(nc, tc, h)
    nc.compile()
    return nc


def _get_nc():
    if "nc" not in _cache:
        _cache["nc"] = _build()
    return _cache["nc"]


def _run(inputs, trace=False):
    nc = _get_nc()
    f32 = np.float32

    def arr(x):
        return np.ascontiguousarray(np.asarray(x, dtype=f32))

    full = {k: arr(v) for k, v in inputs.items()}
    in_maps = []
    for k in range(N_CORES):
        sl = slice(k * MS, (k + 1) * MS)
        in_maps.append({
            "x_in": full["inputs"],
            "h_in": full["h_tm1"],
            "c_in": full["c_tm1"],
            "r_in": full["r_tm1"],
            "m_in": full["m_tm1"][sl],
            "cwu_in": full["c_wu_tm1"][sl],
            "cwlu_in": full["c_wlu_tm1"][sl],
            "cwr_in": full["c_wr_tm1"][sl],
            "wk": full["kernel"],
            "wr": full["recurrent_kernel"],
            "bias": full["bias"].reshape(1, 4 * U),
            "wg": full["write_gate"].reshape(1, 1),
        })
    br = run_bass_kernel_spmd(nc, in_maps, core_ids=list(range(N_CORES)),
                              trace=trace)
    res = br.results
    h_out = res[0]["h_out"]
    c_out = res[0]["c_out"]
    read_out = res[0]["read_out"]
    m_out = np.concatenate([res[k]["m_out"] for k in range(N_CORES)], axis=0)
    cwu_out = np.concatenate([res[k]["cwu_out"] for k in range(N_CORES)], axis=0)
    cwlu_out = np.concatenate([res[k]["cwlu_out"] for k in range(N_CORES)], axis=0)
    cwr_out = np.concatenate([res[k]["cwr_out"] for k in range(N_CORES)], axis=0)
    out = (h_out, c_out, read_out, m_out, cwu_out, cwlu_out, cwr_out)
    return out, br


def kernel(**inputs):
    out, _ = _run(inputs, trace=False)
    return out


# revision 13
# speedup vs baseline: 1.0268x; 1.0248x over previous
"""MANN LSTM cell (scatter_memory) on 8 TRN2 NeuronCores.

Sharding: memory matrix m_tm1 and (M,B) addressing states sharded along the
memory axis (2048 rows/core); LSTM GEMM replicated on every core; one fused
AllGather collective carries the per-core partial read (m_k^T @ c_wr_k) and
per-core column-min of c_wu so every core can finish the write locally.

All matmuls run as float32r (bit-identical storage, single-pass PE) with the
moving dimension >= 256 so the PE runs at 1 cycle/row.  The cosine similarity
is computed transposed (keyT stationary, mT moving, N=512) and transposed back
per 128-row tile for the softmax-over-batch.

kernel(**inputs) takes FULL inputs, returns the FULL output tuple
(h, c, read, m, c_wu, c_wlu, c_wr) exactly like the reference.
"""
import numpy as np

import concourse.bacc as bacc
import concourse.mybir as mybir
from concourse import tile
from concourse.bass_utils import run_bass_kernel_spmd
from concourse.masks import make_identity

N_CORES = 8
B = 64
U = 512
IN_DIM = 512
M_FULL = 16384
MS = M_FULL // N_CORES       # 2048 memory rows per core
NT = MS // 128               # 16 row tiles per core
NG = 4                       # groups of 4 row tiles (512 rows) for cos
KT = IN_DIM // 128           # 4 contraction tiles
UT = U // 128                # 4 unit tiles
DECAY = 0.95
F32 = mybir.dt.float32
F32R = mybir.dt.float32r
EPS = 1e-12

_cache = {}


def _emit(nc, tc, h):
    gp = nc.gpsimd
    ve = nc.vector
    se = nc.scalar
    te = nc.tensor
    dma = nc.sync.dma_start
    dma2 = nc.scalar.dma_start
    X = mybir.AxisListType.X
    Alu = mybir.AluOpType
    Act = mybir.ActivationFunctionType

    def r(ap):
        return ap.bitcast(F32R)

    def f(ap):
        return ap.bitcast(F32)

    with (
        tc.tile_pool(name="const", bufs=1) as const,
        tc.tile_pool(name="big", bufs=1) as big,
        tc.tile_pool(name="sm1", bufs=1) as sm1,
        tc.tile_pool(name="sc2", bufs=2) as sc2,
        tc.tile_pool(name="mtg", bufs=1) as mtgp,
        tc.tile_pool(name="tr_ps", bufs=2, space="PSUM") as trp,
        tc.tile_pool(name="dram", bufs=1, space="DRAM") as dram,
    ):
        id128 = const.tile([128, 128], F32)
        make_identity(nc, id128[:])
        id128r = const.tile([128, 128], F32R)
        ve.tensor_copy(id128r[:], id128[:])

        # ---- big streaming loads issued first so they overlap the LSTM ----
        m_s = big.tile([128, NT, U], F32R)
        m_view = r(h["m_in"].ap().rearrange("(t p) u -> p t u", p=128))
        dma(m_s[:, 0:NT // 2, :], m_view[:, 0:NT // 2, :])
        dma2(m_s[:, NT // 2:NT, :], m_view[:, NT // 2:NT, :])
        cwu_tm1 = big.tile([128, NT, B], F32)
        dma2(cwu_tm1[:], h["cwu_in"].ap().rearrange("(t p) b -> p t b", p=128))
        cwlu_tm1 = big.tile([128, NT, B], F32)
        dma2(cwlu_tm1[:], h["cwlu_in"].ap().rearrange("(t p) b -> p t b", p=128))
        cwr_tm1 = big.tile([128, NT, B], F32)
        dma2(cwr_tm1[:], h["cwr_in"].ap().rearrange("(t p) b -> p t b", p=128))

        # ---- small LSTM state loads ----
        x_s = sm1.tile([B, IN_DIM], F32R)
        dma(x_s[:], r(h["x_in"].ap()))
        h_s = sm1.tile([B, U], F32R)
        dma(h_s[:], r(h["h_in"].ap()))
        c_s = sm1.tile([B, U], F32)
        dma(c_s[:], h["c_in"].ap())
        r_s = sm1.tile([B, U], F32R)
        dma(r_s[:], r(h["r_in"].ap()))
        wg_s = sm1.tile([1, 1], F32)
        dma(wg_s[:], h["wg"].ap())

        # write-gate scalars
        wg_sig = sm1.tile([1, 1], F32)
        se.activation(wg_sig[:], wg_s[:], Act.Sigmoid)
        onemwg = sm1.tile([1, 1], F32)
        ve.tensor_scalar(onemwg[:], wg_sig[:], -1.0, 1.0, Alu.mult, Alu.add)
        wg_b = sm1.tile([128, 1], F32)
        gp.partition_broadcast(wg_b[:], wg_sig[:])
        onemwg_b = sm1.tile([128, 1], F32)
        gp.partition_broadcast(onemwg_b[:], onemwg[:])

        bias_b = sm1.tile([B, 4 * U], F32)
        dma(bias_b[:], h["bias"].ap().broadcast_to((B, 4 * U)))

        # ---- LSTM GEMMs (replicated on every core) ----
        wk_view = h["wk"].ap().rearrange("(k p) j -> p k j", p=128)
        wr_view = h["wr"].ap().rearrange("(k p) j -> p k j", p=128)
        with (
            tc.tile_pool(name="wts", bufs=3) as wts,
            tc.tile_pool(name="g_ps", bufs=1, space="PSUM") as gpsp,
        ):
            # transpose x/h/r to contraction-major for the gate matmuls
            xT = sm1.tile([128, KT, B], F32R)
            hT = sm1.tile([128, KT, B], F32R)
            rT = sm1.tile([128, KT, B], F32R)
            for src, dstT in ((x_s, xT), (h_s, hT), (r_s, rT)):
                for k in range(KT):
                    tp = trp.tile([128, B], F32, name="tp")
                    te.transpose(r(tp[:]), src[:, k * 128:(k + 1) * 128],
                                 id128r[:B, :B])
                    ve.tensor_copy(dstT[:, k, :], tp[:])

            gates = []
            for j in range(4):
                wkc = wts.tile([128, KT, U], F32R, name="wc", tag="wc")
                dma(wkc[:], r(wk_view[:, :, j * U:(j + 1) * U]))
                wrc = wts.tile([128, KT, U], F32R, name="wc", tag="wc")
                dma(wrc[:], r(wr_view[:, :, j * U:(j + 1) * U]))
                gps = gpsp.tile([B, U], F32, name=f"g{j}", tag=f"g{j}")
                gates.append(gps)
                last_k = KT - 1
                for k in range(KT):
                    te.matmul(gps[:], xT[:, k, :], wkc[:, k, :],
                              start=(k == 0), stop=False)
                for k in range(KT):
                    is_last = (j != 0) and (k == last_k)
                    te.matmul(gps[:], hT[:, k, :], wrc[:, k, :],
                              start=False, stop=is_last)
                if j == 0:
                    wrr = wts.tile([128, KT, U], F32R, name="wc", tag="wc")
                    dma(wrr[:], r(wr_view[:, :, 4 * U:5 * U]))
                    for k in range(KT):
                        te.matmul(gps[:], rT[:, k, :], wrr[:, k, :],
                                  start=False, stop=(k == last_k))

            # gate nonlinearities
            def hard_sig(dst, ps, j):
                ve.tensor_tensor(dst[:], ps[:], bias_b[:, j * U:(j + 1) * U], Alu.add)
                ve.tensor_scalar(dst[:], dst[:], 0.2, 0.5, Alu.mult, Alu.add)
                ve.tensor_scalar(dst[:], dst[:], 0.0, 1.0, Alu.max, Alu.min)

            i_g = sm1.tile([B, U], F32)
            f_g = sm1.tile([B, U], F32)
            o_g = sm1.tile([B, U], F32)
            hard_sig(i_g, gates[0], 0)
            hard_sig(f_g, gates[1], 1)
            hard_sig(o_g, gates[3], 3)

            pre_c = sm1.tile([B, U], F32)
            ve.tensor_tensor(pre_c[:], gates[2][:], bias_b[:, 2 * U:3 * U], Alu.add)
            tanh_c = sm1.tile([B, U], F32)
            se.activation(tanh_c[:], pre_c[:], Act.Tanh)

        c_new = sm1.tile([B, U], F32)
        ve.tensor_tensor(c_new[:], f_g[:], c_s[:], Alu.mult)
        t2 = sc2.tile([B, U], F32, name="t2", tag="scr_k")
        ve.tensor_tensor(t2[:], i_g[:], tanh_c[:], Alu.mult)
        ve.tensor_tensor(c_new[:], c_new[:], t2[:], Alu.add)
        dma(h["c_out"].ap(), c_new[:])

        tanh_cn = sm1.tile([B, U], F32)
        se.activation(tanh_cn[:], c_new[:], Act.Tanh)
        h_new = sm1.tile([B, U], F32R)
        ve.tensor_tensor(h_new[:], o_g[:], tanh_cn[:], Alu.mult)
        dma(h["h_out"].ap(), f(h_new[:]))

        # ---- key normalization: n_key = h / max(||h||, eps) ----
        scr_k = sc2.tile([B, U], F32, name="scr_k")
        ksum = sm1.tile([B, 1], F32)
        se.activation(scr_k[:], f(h_new[:]), Act.Square, accum_out=ksum[:])
        ve.tensor_scalar_max(ksum[:], ksum[:], EPS)
        ksq = sm1.tile([B, 1], F32)
        se.activation(ksq[:], ksum[:], Act.Sqrt)
        rkn = sm1.tile([B, 1], F32)
        ve.reciprocal(rkn[:], ksq[:])
        nkey = sm1.tile([B, U], F32R)
        ve.tensor_scalar_mul(nkey[:], f(h_new[:]), rkn[:])

        keyT = sm1.tile([128, UT, B], F32R)
        for u in range(UT):
            tp = trp.tile([128, B], F32, name="tp")
            te.transpose(r(tp[:]), nkey[:, u * 128:(u + 1) * 128], id128r[:B, :B])
            ve.tensor_copy(keyT[:, u, :], tp[:])

        # ---- c_ww = wg*c_wr_tm1 + (1-wg) + c_wlu_tm1 (full width) ----
        cww_all = big.tile([128, NT, B], F32)
        ve.tensor_scalar(cww_all[:], cwr_tm1[:], wg_b[:], onemwg_b[:], Alu.mult, Alu.add)
        ve.tensor_tensor(cww_all[:], cww_all[:], cwlu_tm1[:], Alu.add)

        cwr_all = big.tile([128, NT, B], F32R)
        cwu_all = big.tile([128, NT, B], F32)
        cwlu_all = big.tile([128, NT, B], F32)
        cos_all = big.tile([128, NT, B], F32)

        # ---- 1/||m_row|| for every local row ----
        rmn_all = sm1.tile([128, NT], F32)
        for t in range(NT):
            scr2 = sc2.tile([128, U], F32, name="scr2")
            se.activation(scr2[:], f(m_s[:, t, :]), Act.Square,
                          accum_out=rmn_all[:, t:t + 1])
        ve.tensor_scalar_max(rmn_all[:], rmn_all[:], EPS)
        msq_all = sm1.tile([128, NT], F32)
        se.activation(msq_all[:], rmn_all[:], Act.Sqrt)
        ve.reciprocal(rmn_all[:], msq_all[:])

        with (
            tc.tile_pool(name="cos_ps", bufs=2, space="PSUM") as cosp,
            tc.tile_pool(name="rd_ps", bufs=1, space="PSUM") as rdp,
            tc.tile_pool(name="wr_ps", bufs=2, space="PSUM") as wrp,
        ):
            read_ps = rdp.tile([B, U], F32)

            # ---- cos^T = n_key^T-contraction matmuls, 512-row groups ----
            for g in range(NG):
                mtg = mtgp.tile([128, UT, 512], F32R, name="mtg")
                for ti in range(4):
                    t = g * 4 + ti
                    for u in range(UT):
                        tp = trp.tile([128, 128], F32, name="tp")
                        te.transpose(r(tp[:]), m_s[:, t, u * 128:(u + 1) * 128],
                                     id128r[:])
                        ve.tensor_copy(mtg[:, u, ti * 128:(ti + 1) * 128], tp[:])
                cosT_ps = cosp.tile([B, 512], F32, name="cosT_ps")
                for u in range(UT):
                    te.matmul(cosT_ps[:], keyT[:, u, :], mtg[:, u, :],
                              start=(u == 0), stop=(u == UT - 1))
                cosT_sb = sc2.tile([B, 512], F32, name="cosT_sb")
                ve.tensor_copy(cosT_sb[:], cosT_ps[:])
                for ti in range(4):
                    t = g * 4 + ti
                    tpb = trp.tile([128, B], F32, name="tp")
                    te.transpose(tpb[:], cosT_sb[:, ti * 128:(ti + 1) * 128],
                                 id128[:B, :B])
                    ve.tensor_copy(cos_all[:, t, :], tpb[:])

            # ---- softmax over batch (full width, row-broadcast APs) ----
            def bcast(ap2d):  # (128, NT) -> (128, NT, B) step-0 broadcast
                return ap2d.unsqueeze(2).broadcast_to((128, NT, B))

            ve.tensor_tensor(cos_all[:], cos_all[:], bcast(rmn_all[:]), Alu.mult)
            se.activation(cos_all[:], cos_all[:], Act.Exp)
            rsum = sm1.tile([128, NT], F32)
            ve.tensor_reduce(rsum[:], cos_all[:], X, Alu.add)
            rrec = sm1.tile([128, NT], F32)
            ve.reciprocal(rrec[:], rsum[:])
            ve.tensor_tensor(cwr_all[:], cos_all[:], bcast(rrec[:]), Alu.mult)

            # ---- partial read, accumulated over local row tiles ----
            for t in range(NT):
                te.matmul(read_ps[:], cwr_all[:, t, :], m_s[:, t, :],
                          start=(t == 0), stop=(t == NT - 1))

            # ---- write term s = m + c_ww @ h (independent of the collective) ----
            s_all = big.tile([128, NT, U], F32)
            for t in range(NT):
                tpw = trp.tile([B, 128], F32, name="tp")
                te.transpose(tpw[:], cww_all[:, t, :], id128[:])
                cwwT = sc2.tile([B, 128], F32R, name="cwwT")
                ve.tensor_copy(cwwT[:], tpw[:])
                wr_ps = wrp.tile([128, U], F32, name="wr_ps")
                te.matmul(wr_ps[:], cwwT[:], h_new[:], start=True, stop=True)
                ve.tensor_tensor(s_all[:, t, :], f(m_s[:, t, :]), wr_ps[:], Alu.add)

            # ---- usage update (full width); cwlu_all doubles as scratch ----
            ve.tensor_tensor(cwlu_all[:], f(cwr_all[:]), cww_all[:], Alu.add)
            ve.tensor_scalar(cwu_all[:], cwu_tm1[:], DECAY, None, Alu.mult)
            ve.tensor_tensor(cwu_all[:], cwu_all[:], cwlu_all[:], Alu.add)

            dma2(h["cwr_out"].ap().rearrange("(t p) b -> p t b", p=128),
                 f(cwr_all[:]))
            dma2(h["cwu_out"].ap().rearrange("(t p) b -> p t b", p=128),
                 cwu_all[:])

            # local column-min over the 2048 local rows
            minacc = sm1.tile([128, B], F32)
            ve.tensor_reduce(minacc[:], cwu_all[:].rearrange("p t b -> p b t"), X,
                             Alu.min)
            tpm = trp.tile([B, 128], F32, name="tp")
            te.transpose(tpm[:], minacc[:], id128[:])
            mint = sm1.tile([B, 128], F32)
            ve.tensor_copy(mint[:], tpm[:])
            colmin = sm1.tile([B, 1], F32)
            ve.tensor_reduce(colmin[:], mint[:], X, Alu.min)

            # ---- collective 1: tiny column-min AllGather (gates the tail) ----
            ccm_in = dram.tile([B, 1], F32)
            ccm_out = dram.tile([N_CORES * B, 1], F32, addr_space="Shared")
            dma(ccm_in[:], colmin[:])
            gp.collective_compute(
                "AllGather",
                Alu.bypass,
                ins=[ccm_in[:].opt()],
                outs=[ccm_out[:].opt()],
                replica_groups=[list(range(N_CORES))],
            )
            gathm = sm1.tile([B, N_CORES], F32)
            dma(gathm[:], ccm_out[:].rearrange("(c b) f -> b (c f)", b=B))
            gmin = sm1.tile([B, 1], F32)
            ve.tensor_reduce(gmin[:], gathm[:], X, Alu.min)

            # ---- collective 2: read partials (off the critical path) ----
            cc_in = sm1.tile([B, U], F32)
            ve.tensor_copy(cc_in[:], read_ps[:])
            ccb_in = dram.tile([B, U], F32)
            ccb_out = dram.tile([N_CORES * B, U], F32, addr_space="Shared")
            dma(ccb_in[:], cc_in[:])
            gp.collective_compute(
                "AllGather",
                Alu.bypass,
                ins=[ccb_in[:].opt()],
                outs=[ccb_out[:].opt()],
                replica_groups=[list(range(N_CORES))],
            )
            gath = sm1.tile([B, N_CORES, U], F32)
            dma2(gath[:], ccb_out[:].rearrange("(c b) f -> b c f", b=B))

            read_full = sm1.tile([B, U], F32)
            ve.tensor_tensor(read_full[:], gath[:, 0, :], gath[:, 1, :], Alu.add)
            for c in range(2, N_CORES):
                ve.tensor_tensor(read_full[:], read_full[:], gath[:, c, :], Alu.add)
            dma2(h["read_out"].ap(), read_full[:])

            tpg = trp.tile([1, B], F32, name="tp")
            te.transpose(tpg[:], gmin[:], id128[:B, :B])
            gminrow = sm1.tile([1, B], F32)
            ve.tensor_copy(gminrow[:], tpg[:])
            gmin_b = sm1.tile([128, B], F32)
            gp.partition_broadcast(gmin_b[:], gminrow[:])

            # ---- least-used mask (full width) ----
            gmin_bc = gmin_b[:].unsqueeze(1).broadcast_to((128, NT, B))
            ve.tensor_tensor(cwlu_all[:], cwu_all[:], gmin_bc, Alu.is_le)
            rowflag = sm1.tile([128, NT], F32)
            ve.tensor_reduce(rowflag[:], cwlu_all[:], X, Alu.max)

            # ---- memory write: m_new = s - m*rowflag ----
            m_out_view = h["m_out"].ap().rearrange("(t p) u -> p t u", p=128)
            for t in range(NT):
                corr = sc2.tile([128, U], F32, name="scr2", tag="scr2")
                if t % 2 == 0:
                    se.activation(corr[:], f(m_s[:, t, :]), Act.Copy,
                                  scale=rowflag[:, t:t + 1])
                else:
                    ve.tensor_scalar_mul(corr[:], f(m_s[:, t, :]),
                                         rowflag[:, t:t + 1])
                ve.tensor_tensor(s_all[:, t, :], s_all[:, t, :], corr[:],
                                 Alu.subtract)
                (dma if t % 2 == 0 else dma2)(m_out_view[:, t, :], s_all[:, t, :])

        # ---- remaining outputs ----
        dma(h["cwlu_out"].ap().rearrange("(t p) b -> p t b", p=128), cwlu_all[:])


def _build():
    nc = bacc.Bacc("TRN2", target_bir_lowering=False, debug=False,
                   num_devices=N_CORES)
    h = {}
    h["x_in"] = nc.dram_tensor("x_in", [B, IN_DIM], F32, kind="ExternalInput")
    h["h_in"] = nc.dram_tensor("h_in", [B, U], F32, kind="ExternalInput")
    h["c_in"] = nc.dram_tensor("c_in", [B, U], F32, kind="ExternalInput")
    h["r_in"] = nc.dram_tensor("r_in", [B, U], F32, kind="ExternalInput")
    h["m_in"] = nc.dram_tensor("m_in", [MS, U], F32, kind="ExternalInput")
    h["cwu_in"] = nc.dram_tensor("cwu_in", [MS, B], F32, kind="ExternalInput")
    h["cwlu_in"] = nc.dram_tensor("cwlu_in", [MS, B], F32, kind="ExternalInput")
    h["cwr_in"] = nc.dram_tensor("cwr_in", [MS, B], F32, kind="ExternalInput")
    h["wk"] = nc.dram_tensor("wk", [IN_DIM, 4 * U], F32, kind="ExternalInput")
    h["wr"] = nc.dram_tensor("wr", [U, 5 * U], F32, kind="ExternalInput")
    h["bias"] = nc.dram_tensor("bias", [1, 4 * U], F32, kind="ExternalInput")
    h["wg"] = nc.dram_tensor("wg", [1, 1], F32, kind="ExternalInput")

    h["h_out"] = nc.dram_tensor("h_out", [B, U], F32, kind="ExternalOutput")
    h["c_out"] = nc.dram_tensor("c_out", [B, U], F32, kind="ExternalOutput")
    h["read_out"] = nc.dram_tensor("read_out", [B, U], F32, kind="ExternalOutput")
    h["m_out"] = nc.dram_tensor("m_out", [MS, U], F32, kind="ExternalOutput")
    h["cwu_out"] = nc.dram_tensor("cwu_out", [MS, B], F32, kind="ExternalOutput")
    h["cwlu_out"] = nc.dram_tensor("cwlu_out", [MS, B], F32, kind="ExternalOutput")
    h["cwr_out"] = nc.dram_tensor("cwr_out", [MS, B], F32, kind="ExternalOutput")

    with tile.TileContext(nc) as tc:
        _emit(nc, tc, h)
    nc.compile()
    return nc


def _get_nc():
    if "nc" not in _cache:
        _cache["nc"] = _build()
    return _cache["nc"]


def _run(inputs, trace=False):
    nc = _get_nc()
    f32 = np.float32

    def arr(x):
        return np.ascontiguousarray(np.asarray(x, dtype=f32))

    full = {k: arr(v) for k, v in inputs.items()}
    in_maps = []
    for k in range(N_CORES):
        sl = slice(k * MS, (k + 1) * MS)
        in_maps.append({
            "x_in": full["inputs"],
            "h_in": full["h_tm1"],
            "c_in": full["c_tm1"],
            "r_in": full["r_tm1"],
            "m_in": full["m_tm1"][sl],
            "cwu_in": full["c_wu_tm1"][sl],
            "cwlu_in": full["c_wlu_tm1"][sl],
            "cwr_in": full["c_wr_tm1"][sl],
            "wk": full["kernel"],
            "wr": full["recurrent_kernel"],
            "bias": full["bias"].reshape(1, 4 * U),
            "wg": full["write_gate"].reshape(1, 1),
        })
    br = run_bass_kernel_spmd(nc, in_maps, core_ids=list(range(N_CORES)),
                              trace=trace)
    res = br.results
    h_out = res[0]["h_out"]
    c_out = res[0]["c_out"]
    read_out = res[0]["read_out"]
    m_out = np.concatenate([res[k]["m_out"] for k in range(N_CORES)], axis=0)
    cwu_out = np.concatenate([res[k]["cwu_out"] for k in range(N_CORES)], axis=0)
    cwlu_out = np.concatenate([res[k]["cwlu_out"] for k in range(N_CORES)], axis=0)
    cwr_out = np.concatenate([res[k]["cwr_out"] for k in range(N_CORES)], axis=0)
    out = (h_out, c_out, read_out, m_out, cwu_out, cwlu_out, cwr_out)
    return out, br


def kernel(**inputs):
    out, _ = _run(inputs, trace=False)
    return out


# revision 14
# speedup vs baseline: 1.0408x; 1.0136x over previous
"""MANN LSTM cell (scatter_memory) on 8 TRN2 NeuronCores.

Sharding: memory matrix m_tm1 and (M,B) addressing states sharded along the
memory axis (2048 rows/core); LSTM GEMM replicated on every core; one fused
AllGather collective carries the per-core partial read (m_k^T @ c_wr_k) and
per-core column-min of c_wu so every core can finish the write locally.

All matmuls run as float32r (bit-identical storage, single-pass PE) with the
moving dimension >= 256 so the PE runs at 1 cycle/row.  The cosine similarity
is computed transposed (keyT stationary, mT moving, N=512) and transposed back
per 128-row tile for the softmax-over-batch.

kernel(**inputs) takes FULL inputs, returns the FULL output tuple
(h, c, read, m, c_wu, c_wlu, c_wr) exactly like the reference.
"""
import numpy as np

import concourse.bacc as bacc
import concourse.mybir as mybir
from concourse import tile
from concourse.bass_utils import run_bass_kernel_spmd
from concourse.masks import make_identity

N_CORES = 8
B = 64
U = 512
IN_DIM = 512
M_FULL = 16384
MS = M_FULL // N_CORES       # 2048 memory rows per core
NT = MS // 128               # 16 row tiles per core
NG = 4                       # groups of 4 row tiles (512 rows) for cos
KT = IN_DIM // 128           # 4 contraction tiles
UT = U // 128                # 4 unit tiles
DECAY = 0.95
F32 = mybir.dt.float32
F32R = mybir.dt.float32r
EPS = 1e-12

_cache = {}


def _emit(nc, tc, h):
    gp = nc.gpsimd
    ve = nc.vector
    se = nc.scalar
    te = nc.tensor
    dma = nc.sync.dma_start
    dma2 = nc.scalar.dma_start
    X = mybir.AxisListType.X
    Alu = mybir.AluOpType
    Act = mybir.ActivationFunctionType

    def r(ap):
        return ap.bitcast(F32R)

    def f(ap):
        return ap.bitcast(F32)

    with (
        tc.tile_pool(name="const", bufs=1) as const,
        tc.tile_pool(name="big", bufs=1) as big,
        tc.tile_pool(name="sm1", bufs=1) as sm1,
        tc.tile_pool(name="sc2", bufs=2) as sc2,
        tc.tile_pool(name="mtg", bufs=1) as mtgp,
        tc.tile_pool(name="tr_ps", bufs=2, space="PSUM") as trp,
        tc.tile_pool(name="dram", bufs=1, space="DRAM") as dram,
    ):
        id128 = const.tile([128, 128], F32)
        make_identity(nc, id128[:])
        id128r = const.tile([128, 128], F32R)
        ve.tensor_copy(id128r[:], id128[:])

        # ---- big streaming loads issued first so they overlap the LSTM ----
        m_s = big.tile([128, NT, U], F32R)
        m_view = r(h["m_in"].ap().rearrange("(t p) u -> p t u", p=128))
        dma(m_s[:, 0:NT // 2, :], m_view[:, 0:NT // 2, :])
        dma2(m_s[:, NT // 2:NT, :], m_view[:, NT // 2:NT, :])
        cwu_tm1 = big.tile([128, NT, B], F32)
        dma2(cwu_tm1[:], h["cwu_in"].ap().rearrange("(t p) b -> p t b", p=128))
        cwlu_tm1 = big.tile([128, NT, B], F32)
        dma2(cwlu_tm1[:], h["cwlu_in"].ap().rearrange("(t p) b -> p t b", p=128))
        cwr_tm1 = big.tile([128, NT, B], F32)
        dma2(cwr_tm1[:], h["cwr_in"].ap().rearrange("(t p) b -> p t b", p=128))

        # ---- small LSTM state loads ----
        x_s = sm1.tile([B, IN_DIM], F32R)
        dma(x_s[:], r(h["x_in"].ap()))
        h_s = sm1.tile([B, U], F32R)
        dma(h_s[:], r(h["h_in"].ap()))
        c_s = sm1.tile([B, U], F32)
        dma(c_s[:], h["c_in"].ap())
        r_s = sm1.tile([B, U], F32R)
        dma(r_s[:], r(h["r_in"].ap()))
        wg_s = sm1.tile([1, 1], F32)
        dma(wg_s[:], h["wg"].ap())

        # write-gate scalars
        wg_sig = sm1.tile([1, 1], F32)
        se.activation(wg_sig[:], wg_s[:], Act.Sigmoid)
        onemwg = sm1.tile([1, 1], F32)
        ve.tensor_scalar(onemwg[:], wg_sig[:], -1.0, 1.0, Alu.mult, Alu.add)
        wg_b = sm1.tile([128, 1], F32)
        gp.partition_broadcast(wg_b[:], wg_sig[:])
        onemwg_b = sm1.tile([128, 1], F32)
        gp.partition_broadcast(onemwg_b[:], onemwg[:])

        bias_b = sm1.tile([B, 4 * U], F32)
        dma(bias_b[:], h["bias"].ap().broadcast_to((B, 4 * U)))

        # ---- LSTM GEMMs (replicated on every core) ----
        wk_view = h["wk"].ap().rearrange("(k p) j -> p k j", p=128)
        wr_view = h["wr"].ap().rearrange("(k p) j -> p k j", p=128)
        with (
            tc.tile_pool(name="wts", bufs=3) as wts,
            tc.tile_pool(name="g_ps", bufs=1, space="PSUM") as gpsp,
        ):
            # transpose x/h/r to contraction-major for the gate matmuls
            xT = sm1.tile([128, KT, B], F32R)
            hT = sm1.tile([128, KT, B], F32R)
            rT = sm1.tile([128, KT, B], F32R)
            for src, dstT in ((x_s, xT), (h_s, hT), (r_s, rT)):
                for k in range(KT):
                    tp = trp.tile([128, B], F32, name="tp")
                    te.transpose(r(tp[:]), src[:, k * 128:(k + 1) * 128],
                                 id128r[:B, :B])
                    ve.tensor_copy(dstT[:, k, :], tp[:])

            gates = []
            for j in range(4):
                wkc = wts.tile([128, KT, U], F32R, name="wc", tag="wc")
                dma(wkc[:], r(wk_view[:, :, j * U:(j + 1) * U]))
                wrc = wts.tile([128, KT, U], F32R, name="wc", tag="wc")
                dma(wrc[:], r(wr_view[:, :, j * U:(j + 1) * U]))
                gps = gpsp.tile([B, U], F32, name=f"g{j}", tag=f"g{j}")
                gates.append(gps)
                last_k = KT - 1
                for k in range(KT):
                    te.matmul(gps[:], xT[:, k, :], wkc[:, k, :],
                              start=(k == 0), stop=False)
                for k in range(KT):
                    is_last = (j != 0) and (k == last_k)
                    te.matmul(gps[:], hT[:, k, :], wrc[:, k, :],
                              start=False, stop=is_last)
                if j == 0:
                    wrr = wts.tile([128, KT, U], F32R, name="wc", tag="wc")
                    dma(wrr[:], r(wr_view[:, :, 4 * U:5 * U]))
                    for k in range(KT):
                        te.matmul(gps[:], rT[:, k, :], wrr[:, k, :],
                                  start=False, stop=(k == last_k))

            # gate nonlinearities
            def hard_sig(dst, ps, j):
                ve.tensor_tensor(dst[:], ps[:], bias_b[:, j * U:(j + 1) * U], Alu.add)
                ve.tensor_scalar(dst[:], dst[:], 0.2, 0.5, Alu.mult, Alu.add)
                ve.tensor_scalar(dst[:], dst[:], 0.0, 1.0, Alu.max, Alu.min)

            i_g = sm1.tile([B, U], F32)
            f_g = sm1.tile([B, U], F32)
            o_g = sm1.tile([B, U], F32)
            hard_sig(i_g, gates[0], 0)
            hard_sig(f_g, gates[1], 1)
            hard_sig(o_g, gates[3], 3)

            pre_c = sm1.tile([B, U], F32)
            ve.tensor_tensor(pre_c[:], gates[2][:], bias_b[:, 2 * U:3 * U], Alu.add)
            tanh_c = sm1.tile([B, U], F32)
            se.activation(tanh_c[:], pre_c[:], Act.Tanh)

        c_new = sm1.tile([B, U], F32)
        ve.tensor_tensor(c_new[:], f_g[:], c_s[:], Alu.mult)
        t2 = sc2.tile([B, U], F32, name="t2", tag="scr_k")
        ve.tensor_tensor(t2[:], i_g[:], tanh_c[:], Alu.mult)
        ve.tensor_tensor(c_new[:], c_new[:], t2[:], Alu.add)
        dma(h["c_out"].ap(), c_new[:])

        tanh_cn = sm1.tile([B, U], F32)
        se.activation(tanh_cn[:], c_new[:], Act.Tanh)
        h_new = sm1.tile([B, U], F32R)
        ve.tensor_tensor(h_new[:], o_g[:], tanh_cn[:], Alu.mult)
        dma(h["h_out"].ap(), f(h_new[:]))

        # ---- key normalization: n_key = h / max(||h||, eps) ----
        scr_k = sc2.tile([B, U], F32, name="scr_k")
        ksum = sm1.tile([B, 1], F32)
        se.activation(scr_k[:], f(h_new[:]), Act.Square, accum_out=ksum[:])
        ve.tensor_scalar_max(ksum[:], ksum[:], EPS)
        ksq = sm1.tile([B, 1], F32)
        se.activation(ksq[:], ksum[:], Act.Sqrt)
        rkn = sm1.tile([B, 1], F32)
        ve.reciprocal(rkn[:], ksq[:])
        nkey = sm1.tile([B, U], F32R)
        ve.tensor_scalar_mul(nkey[:], f(h_new[:]), rkn[:])

        keyT = sm1.tile([128, UT, B], F32R)
        for u in range(UT):
            tp = trp.tile([128, B], F32, name="tp")
            te.transpose(r(tp[:]), nkey[:, u * 128:(u + 1) * 128], id128r[:B, :B])
            ve.tensor_copy(keyT[:, u, :], tp[:])

        # ---- c_ww = wg*c_wr_tm1 + (1-wg) + c_wlu_tm1 (full width) ----
        cww_all = big.tile([128, NT, B], F32)
        ve.tensor_scalar(cww_all[:], cwr_tm1[:], wg_b[:], onemwg_b[:], Alu.mult, Alu.add)
        ve.tensor_tensor(cww_all[:], cww_all[:], cwlu_tm1[:], Alu.add)

        cwr_all = big.tile([128, NT, B], F32R)
        cwu_all = big.tile([128, NT, B], F32)
        cwlu_all = big.tile([128, NT, B], F32)
        cos_all = big.tile([128, NT, B], F32)

        # ---- 1/||m_row|| for every local row ----
        rmn_all = sm1.tile([128, NT], F32)
        for t in range(NT):
            scr2 = sc2.tile([128, U], F32, name="scr2")
            se.activation(scr2[:], f(m_s[:, t, :]), Act.Square,
                          accum_out=rmn_all[:, t:t + 1])
        ve.tensor_scalar_max(rmn_all[:], rmn_all[:], EPS)
        msq_all = sm1.tile([128, NT], F32)
        se.activation(msq_all[:], rmn_all[:], Act.Sqrt)
        ve.reciprocal(rmn_all[:], msq_all[:])

        with (
            tc.tile_pool(name="cos_ps", bufs=2, space="PSUM") as cosp,
            tc.tile_pool(name="rd_ps", bufs=1, space="PSUM") as rdp,
            tc.tile_pool(name="wr_ps", bufs=2, space="PSUM") as wrp,
        ):
            read_ps = rdp.tile([B, U], F32)

            # ---- cos^T = n_key^T-contraction matmuls, 512-row groups ----
            for g in range(NG):
                mtg = mtgp.tile([128, UT, 512], F32R, name="mtg")
                for ti in range(4):
                    t = g * 4 + ti
                    for u in range(UT):
                        tp = trp.tile([128, 128], F32, name="tp")
                        te.transpose(r(tp[:]), m_s[:, t, u * 128:(u + 1) * 128],
                                     id128r[:])
                        ve.tensor_copy(mtg[:, u, ti * 128:(ti + 1) * 128], tp[:])
                cosT_ps = cosp.tile([B, 512], F32, name="cosT_ps")
                for u in range(UT):
                    te.matmul(cosT_ps[:], keyT[:, u, :], mtg[:, u, :],
                              start=(u == 0), stop=(u == UT - 1))
                cosT_sb = sc2.tile([B, 512], F32, name="cosT_sb")
                ve.tensor_copy(cosT_sb[:], cosT_ps[:])
                for ti in range(4):
                    t = g * 4 + ti
                    tpb = trp.tile([128, B], F32, name="tp")
                    te.transpose(tpb[:], cosT_sb[:, ti * 128:(ti + 1) * 128],
                                 id128[:B, :B])
                    ve.tensor_copy(cos_all[:, t, :], tpb[:])

            # ---- softmax over batch (full width, row-broadcast APs) ----
            def bcast(ap2d):  # (128, NT) -> (128, NT, B) step-0 broadcast
                return ap2d.unsqueeze(2).broadcast_to((128, NT, B))

            ve.tensor_tensor(cos_all[:], cos_all[:], bcast(rmn_all[:]), Alu.mult)
            se.activation(cos_all[:], cos_all[:], Act.Exp)
            rsum = sm1.tile([128, NT], F32)
            ve.tensor_reduce(rsum[:], cos_all[:], X, Alu.add)
            rrec = sm1.tile([128, NT], F32)
            ve.reciprocal(rrec[:], rsum[:])
            ve.tensor_tensor(cwr_all[:], cos_all[:], bcast(rrec[:]), Alu.mult)

            # ---- usage update (full width); cwlu_all doubles as scratch ----
            ve.tensor_tensor(cwlu_all[:], f(cwr_all[:]), cww_all[:], Alu.add)
            ve.tensor_scalar(cwu_all[:], cwu_tm1[:], DECAY, None, Alu.mult)
            ve.tensor_tensor(cwu_all[:], cwu_all[:], cwlu_all[:], Alu.add)

            dma2(h["cwr_out"].ap().rearrange("(t p) b -> p t b", p=128),
                 f(cwr_all[:]))
            dma2(h["cwu_out"].ap().rearrange("(t p) b -> p t b", p=128),
                 cwu_all[:])

            # local column-min over the 2048 local rows
            minacc = sm1.tile([128, B], F32)
            ve.tensor_reduce(minacc[:], cwu_all[:].rearrange("p t b -> p b t"), X,
                             Alu.min)
            tpm = trp.tile([B, 128], F32, name="tp")
            te.transpose(tpm[:], minacc[:], id128[:])
            mint = sm1.tile([B, 128], F32)
            ve.tensor_copy(mint[:], tpm[:])
            colmin = sm1.tile([B, 1], F32)
            ve.tensor_reduce(colmin[:], mint[:], X, Alu.min)

            # ---- collective 1: tiny column-min AllGather (gates the tail) ----
            ccm_in = dram.tile([B, 1], F32)
            ccm_out = dram.tile([N_CORES * B, 1], F32, addr_space="Shared")
            dma(ccm_in[:], colmin[:])
            gp.collective_compute(
                "AllGather",
                Alu.bypass,
                ins=[ccm_in[:].opt()],
                outs=[ccm_out[:].opt()],
                replica_groups=[list(range(N_CORES))],
            )
            gathm = sm1.tile([B, N_CORES], F32)
            dma(gathm[:], ccm_out[:].rearrange("(c b) f -> b (c f)", b=B))
            gmin = sm1.tile([B, 1], F32)
            ve.tensor_reduce(gmin[:], gathm[:], X, Alu.min)

            # ---- partial read, accumulated over local row tiles ----
            for t in range(NT):
                te.matmul(read_ps[:], cwr_all[:, t, :], m_s[:, t, :],
                          start=(t == 0), stop=(t == NT - 1))

            # ---- write term s = m + c_ww @ h (independent of the collective) ----
            s_all = big.tile([128, NT, U], F32)
            for t in range(NT):
                tpw = trp.tile([B, 128], F32, name="tp")
                te.transpose(tpw[:], cww_all[:, t, :], id128[:])
                cwwT = sc2.tile([B, 128], F32R, name="cwwT")
                ve.tensor_copy(cwwT[:], tpw[:])
                wr_ps = wrp.tile([128, U], F32, name="wr_ps")
                te.matmul(wr_ps[:], cwwT[:], h_new[:], start=True, stop=True)
                ve.tensor_tensor(s_all[:, t, :], f(m_s[:, t, :]), wr_ps[:], Alu.add)

            # ---- collective 2: read partials (off the critical path) ----
            cc_in = sm1.tile([B, U], F32)
            ve.tensor_copy(cc_in[:], read_ps[:])
            ccb_in = dram.tile([B, U], F32)
            ccb_out = dram.tile([N_CORES * B, U], F32, addr_space="Shared")
            dma(ccb_in[:], cc_in[:])
            gp.collective_compute(
                "AllGather",
                Alu.bypass,
                ins=[ccb_in[:].opt()],
                outs=[ccb_out[:].opt()],
                replica_groups=[list(range(N_CORES))],
            )
            gath = sm1.tile([B, N_CORES, U], F32)
            dma2(gath[:], ccb_out[:].rearrange("(c b) f -> b c f", b=B))

            read_full = sm1.tile([B, U], F32)
            ve.tensor_tensor(read_full[:], gath[:, 0, :], gath[:, 1, :], Alu.add)
            for c in range(2, N_CORES):
                ve.tensor_tensor(read_full[:], read_full[:], gath[:, c, :], Alu.add)
            dma2(h["read_out"].ap(), read_full[:])

            tpg = trp.tile([1, B], F32, name="tp")
            te.transpose(tpg[:], gmin[:], id128[:B, :B])
            gminrow = sm1.tile([1, B], F32)
            ve.tensor_copy(gminrow[:], tpg[:])
            gmin_b = sm1.tile([128, B], F32)
            gp.partition_broadcast(gmin_b[:], gminrow[:])

            # ---- least-used mask (full width) ----
            gmin_bc = gmin_b[:].unsqueeze(1).broadcast_to((128, NT, B))
            ve.tensor_tensor(cwlu_all[:], cwu_all[:], gmin_bc, Alu.is_le)
            rowflag = sm1.tile([128, NT], F32)
            ve.tensor_reduce(rowflag[:], cwlu_all[:], X, Alu.max)

            # ---- memory write: m_new = s - m*rowflag ----
            m_out_view = h["m_out"].ap().rearrange("(t p) u -> p t u", p=128)
            for t in range(NT):
                corr = sc2.tile([128, U], F32, name="scr2", tag="scr2")
                if t % 2 == 0:
                    se.activation(corr[:], f(m_s[:, t, :]), Act.Copy,
                                  scale=rowflag[:, t:t + 1])
                else:
                    ve.tensor_scalar_mul(corr[:], f(m_s[:, t, :]),
                                         rowflag[:, t:t + 1])
                ve.tensor_tensor(s_all[:, t, :], s_all[:, t, :], corr[:],
                                 Alu.subtract)
                (dma if t % 2 == 0 else dma2)(m_out_view[:, t, :], s_all[:, t, :])

        # ---- remaining outputs ----
        dma(h["cwlu_out"].ap().rearrange("(t p) b -> p t b", p=128), cwlu_all[:])


def _build():
    nc = bacc.Bacc("TRN2", target_bir_lowering=False, debug=False,
                   num_devices=N_CORES)
    h = {}
    h["x_in"] = nc.dram_tensor("x_in", [B, IN_DIM], F32, kind="ExternalInput")
    h["h_in"] = nc.dram_tensor("h_in", [B, U], F32, kind="ExternalInput")
    h["c_in"] = nc.dram_tensor("c_in", [B, U], F32, kind="ExternalInput")
    h["r_in"] = nc.dram_tensor("r_in", [B, U], F32, kind="ExternalInput")
    h["m_in"] = nc.dram_tensor("m_in", [MS, U], F32, kind="ExternalInput")
    h["cwu_in"] = nc.dram_tensor("cwu_in", [MS, B], F32, kind="ExternalInput")
    h["cwlu_in"] = nc.dram_tensor("cwlu_in", [MS, B], F32, kind="ExternalInput")
    h["cwr_in"] = nc.dram_tensor("cwr_in", [MS, B], F32, kind="ExternalInput")
    h["wk"] = nc.dram_tensor("wk", [IN_DIM, 4 * U], F32, kind="ExternalInput")
    h["wr"] = nc.dram_tensor("wr", [U, 5 * U], F32, kind="ExternalInput")
    h["bias"] = nc.dram_tensor("bias", [1, 4 * U], F32, kind="ExternalInput")
    h["wg"] = nc.dram_tensor("wg", [1, 1], F32, kind="ExternalInput")

    h["h_out"] = nc.dram_tensor("h_out", [B, U], F32, kind="ExternalOutput")
    h["c_out"] = nc.dram_tensor("c_out", [B, U], F32, kind="ExternalOutput")
    h["read_out"] = nc.dram_tensor("read_out", [B, U], F32, kind="ExternalOutput")
    h["m_out"] = nc.dram_tensor("m_out", [MS, U], F32, kind="ExternalOutput")
    h["cwu_out"] = nc.dram_tensor("cwu_out", [MS, B], F32, kind="ExternalOutput")
    h["cwlu_out"] = nc.dram_tensor("cwlu_out", [MS, B], F32, kind="ExternalOutput")
    h["cwr_out"] = nc.dram_tensor("cwr_out", [MS, B], F32, kind="ExternalOutput")

    with tile.TileContext(nc) as tc:
        _emit(nc, tc, h)
    nc.compile()
    return nc


def _get_nc():
    if "nc" not in _cache:
        _cache["nc"] = _build()
    return _cache["nc"]


def _run(inputs, trace=False):
    nc = _get_nc()
    f32 = np.float32

    def arr(x):
        return np.ascontiguousarray(np.asarray(x, dtype=f32))

    full = {k: arr(v) for k, v in inputs.items()}
    in_maps = []
    for k in range(N_CORES):
        sl = slice(k * MS, (k + 1) * MS)
        in_maps.append({
            "x_in": full["inputs"],
            "h_in": full["h_tm1"],
            "c_in": full["c_tm1"],
            "r_in": full["r_tm1"],
            "m_in": full["m_tm1"][sl],
            "cwu_in": full["c_wu_tm1"][sl],
            "cwlu_in": full["c_wlu_tm1"][sl],
            "cwr_in": full["c_wr_tm1"][sl],
            "wk": full["kernel"],
            "wr": full["recurrent_kernel"],
            "bias": full["bias"].reshape(1, 4 * U),
            "wg": full["write_gate"].reshape(1, 1),
        })
    br = run_bass_kernel_spmd(nc, in_maps, core_ids=list(range(N_CORES)),
                              trace=trace)
    res = br.results
    h_out = res[0]["h_out"]
    c_out = res[0]["c_out"]
    read_out = res[0]["read_out"]
    m_out = np.concatenate([res[k]["m_out"] for k in range(N_CORES)], axis=0)
    cwu_out = np.concatenate([res[k]["cwu_out"] for k in range(N_CORES)], axis=0)
    cwlu_out = np.concatenate([res[k]["cwlu_out"] for k in range(N_CORES)], axis=0)
    cwr_out = np.concatenate([res[k]["cwr_out"] for k in range(N_CORES)], axis=0)
    out = (h_out, c_out, read_out, m_out, cwu_out, cwlu_out, cwr_out)
    return out, br


def kernel(**inputs):
    out, _ = _run(inputs, trace=False)
    return out


# revision 16
# speedup vs baseline: 1.0827x; 1.0403x over previous
"""MANN LSTM cell (scatter_memory) on 8 TRN2 NeuronCores.

Sharding: memory matrix m_tm1 and (M,B) addressing states sharded along the
memory axis (2048 rows/core); LSTM GEMM replicated on every core; one fused
AllGather collective carries the per-core partial read (m_k^T @ c_wr_k) and
per-core column-min of c_wu so every core can finish the write locally.

All matmuls run as float32r (bit-identical storage, single-pass PE) with the
moving dimension >= 256 so the PE runs at 1 cycle/row.  The cosine similarity
is computed transposed (keyT stationary, mT moving, N=512) and transposed back
per 128-row tile for the softmax-over-batch.

kernel(**inputs) takes FULL inputs, returns the FULL output tuple
(h, c, read, m, c_wu, c_wlu, c_wr) exactly like the reference.
"""
import numpy as np

import concourse.bacc as bacc
import concourse.mybir as mybir
from concourse import tile
from concourse.bass_utils import run_bass_kernel_spmd
from concourse.masks import make_identity

N_CORES = 8
B = 64
U = 512
IN_DIM = 512
M_FULL = 16384
MS = M_FULL // N_CORES       # 2048 memory rows per core
NT = MS // 128               # 16 row tiles per core
NG = 4                       # groups of 4 row tiles (512 rows) for cos
KT = IN_DIM // 128           # 4 contraction tiles
UT = U // 128                # 4 unit tiles
DECAY = 0.95
F32 = mybir.dt.float32
F32R = mybir.dt.float32r
EPS = 1e-12

_cache = {}


def _emit(nc, tc, h):
    gp = nc.gpsimd
    ve = nc.vector
    se = nc.scalar
    te = nc.tensor
    dma = nc.sync.dma_start
    dma2 = nc.scalar.dma_start
    X = mybir.AxisListType.X
    Alu = mybir.AluOpType
    Act = mybir.ActivationFunctionType

    def r(ap):
        return ap.bitcast(F32R)

    def f(ap):
        return ap.bitcast(F32)

    with (
        tc.tile_pool(name="const", bufs=1) as const,
        tc.tile_pool(name="big", bufs=1) as big,
        tc.tile_pool(name="sm1", bufs=1) as sm1,
        tc.tile_pool(name="sc2", bufs=2) as sc2,
        tc.tile_pool(name="mtg", bufs=2) as mtgp,
        tc.tile_pool(name="tr_ps", bufs=3, space="PSUM") as trp,
        tc.tile_pool(name="dram", bufs=1, space="DRAM") as dram,
    ):
        id128 = const.tile([128, 128], F32)
        make_identity(nc, id128[:])
        id128r = const.tile([128, 128], F32R)
        ve.tensor_copy(id128r[:], id128[:])

        # ---- big streaming loads issued first so they overlap the LSTM ----
        m_s = big.tile([128, NT, U], F32R)
        m_view = r(h["m_in"].ap().rearrange("(t p) u -> p t u", p=128))
        dma(m_s[:, 0:NT // 2, :], m_view[:, 0:NT // 2, :])
        dma2(m_s[:, NT // 2:NT, :], m_view[:, NT // 2:NT, :])
        cwu_tm1 = big.tile([128, NT, B], F32)
        dma2(cwu_tm1[:], h["cwu_in"].ap().rearrange("(t p) b -> p t b", p=128))
        cwlu_tm1 = big.tile([128, NT, B], F32)
        dma2(cwlu_tm1[:], h["cwlu_in"].ap().rearrange("(t p) b -> p t b", p=128))
        cwr_tm1 = big.tile([128, NT, B], F32)
        dma2(cwr_tm1[:], h["cwr_in"].ap().rearrange("(t p) b -> p t b", p=128))

        # ---- small LSTM state loads ----
        x_s = sm1.tile([B, IN_DIM], F32R)
        dma(x_s[:], r(h["x_in"].ap()))
        h_s = sm1.tile([B, U], F32R)
        dma(h_s[:], r(h["h_in"].ap()))
        c_s = sm1.tile([B, U], F32)
        dma(c_s[:], h["c_in"].ap())
        r_s = sm1.tile([B, U], F32R)
        dma(r_s[:], r(h["r_in"].ap()))
        wg_s = sm1.tile([1, 1], F32)
        dma(wg_s[:], h["wg"].ap())

        # write-gate scalars
        wg_sig = sm1.tile([1, 1], F32)
        se.activation(wg_sig[:], wg_s[:], Act.Sigmoid)
        onemwg = sm1.tile([1, 1], F32)
        ve.tensor_scalar(onemwg[:], wg_sig[:], -1.0, 1.0, Alu.mult, Alu.add)
        wg_b = sm1.tile([128, 1], F32)
        gp.partition_broadcast(wg_b[:], wg_sig[:])
        onemwg_b = sm1.tile([128, 1], F32)
        gp.partition_broadcast(onemwg_b[:], onemwg[:])

        bias_b = sm1.tile([B, 4 * U], F32)
        dma(bias_b[:], h["bias"].ap().broadcast_to((B, 4 * U)))

        # ---- LSTM GEMMs (replicated on every core) ----
        wk_view = h["wk"].ap().rearrange("(k p) j -> p k j", p=128)
        wr_view = h["wr"].ap().rearrange("(k p) j -> p k j", p=128)
        with (
            tc.tile_pool(name="wts", bufs=3) as wts,
            tc.tile_pool(name="g_ps", bufs=1, space="PSUM") as gpsp,
        ):
            # transpose x/h/r to contraction-major for the gate matmuls
            xT = sm1.tile([128, KT, B], F32R)
            hT = sm1.tile([128, KT, B], F32R)
            rT = sm1.tile([128, KT, B], F32R)
            for src, dstT in ((x_s, xT), (h_s, hT), (r_s, rT)):
                for k in range(KT):
                    tp = trp.tile([128, B], F32, name="tp")
                    te.transpose(r(tp[:]), src[:, k * 128:(k + 1) * 128],
                                 id128r[:B, :B])
                    ve.tensor_copy(dstT[:, k, :], tp[:])

            gates = []
            for j in range(4):
                wkc = wts.tile([128, KT, U], F32R, name="wc", tag="wc")
                dma(wkc[:], r(wk_view[:, :, j * U:(j + 1) * U]))
                wrc = wts.tile([128, KT, U], F32R, name="wc", tag="wc")
                dma(wrc[:], r(wr_view[:, :, j * U:(j + 1) * U]))
                gps = gpsp.tile([B, U], F32, name=f"g{j}", tag=f"g{j}")
                gates.append(gps)
                last_k = KT - 1
                for k in range(KT):
                    te.matmul(gps[:], xT[:, k, :], wkc[:, k, :],
                              start=(k == 0), stop=False)
                for k in range(KT):
                    is_last = (j != 0) and (k == last_k)
                    te.matmul(gps[:], hT[:, k, :], wrc[:, k, :],
                              start=False, stop=is_last)
                if j == 0:
                    wrr = wts.tile([128, KT, U], F32R, name="wc", tag="wc")
                    dma(wrr[:], r(wr_view[:, :, 4 * U:5 * U]))
                    for k in range(KT):
                        te.matmul(gps[:], rT[:, k, :], wrr[:, k, :],
                                  start=False, stop=(k == last_k))

            # gate nonlinearities
            def hard_sig(dst, ps, j):
                ve.tensor_tensor(dst[:], ps[:], bias_b[:, j * U:(j + 1) * U], Alu.add)
                ve.tensor_scalar(dst[:], dst[:], 0.2, 0.5, Alu.mult, Alu.add)
                ve.tensor_scalar(dst[:], dst[:], 0.0, 1.0, Alu.max, Alu.min)

            i_g = sm1.tile([B, U], F32)
            f_g = sm1.tile([B, U], F32)
            o_g = sm1.tile([B, U], F32)
            hard_sig(i_g, gates[0], 0)
            hard_sig(f_g, gates[1], 1)
            hard_sig(o_g, gates[3], 3)

            pre_c = sm1.tile([B, U], F32)
            ve.tensor_tensor(pre_c[:], gates[2][:], bias_b[:, 2 * U:3 * U], Alu.add)
            tanh_c = sm1.tile([B, U], F32)
            se.activation(tanh_c[:], pre_c[:], Act.Tanh)

        c_new = sm1.tile([B, U], F32)
        ve.tensor_tensor(c_new[:], f_g[:], c_s[:], Alu.mult)
        t2 = sc2.tile([B, U], F32, name="t2", tag="scr_k")
        ve.tensor_tensor(t2[:], i_g[:], tanh_c[:], Alu.mult)
        ve.tensor_tensor(c_new[:], c_new[:], t2[:], Alu.add)
        dma(h["c_out"].ap(), c_new[:])

        tanh_cn = sm1.tile([B, U], F32)
        se.activation(tanh_cn[:], c_new[:], Act.Tanh)
        h_new = sm1.tile([B, U], F32R)
        ve.tensor_tensor(h_new[:], o_g[:], tanh_cn[:], Alu.mult)
        dma(h["h_out"].ap(), f(h_new[:]))

        # ---- key normalization: n_key = h / max(||h||, eps) ----
        scr_k = sc2.tile([B, U], F32, name="scr_k")
        ksum = sm1.tile([B, 1], F32)
        se.activation(scr_k[:], f(h_new[:]), Act.Square, accum_out=ksum[:])
        ve.tensor_scalar_max(ksum[:], ksum[:], EPS)
        ksq = sm1.tile([B, 1], F32)
        se.activation(ksq[:], ksum[:], Act.Sqrt)
        rkn = sm1.tile([B, 1], F32)
        ve.reciprocal(rkn[:], ksq[:])
        nkey = sm1.tile([B, U], F32R)
        ve.tensor_scalar_mul(nkey[:], f(h_new[:]), rkn[:])

        keyT = sm1.tile([128, UT, B], F32R)
        for u in range(UT):
            tp = trp.tile([128, B], F32, name="tp")
            te.transpose(r(tp[:]), nkey[:, u * 128:(u + 1) * 128], id128r[:B, :B])
            ve.tensor_copy(keyT[:, u, :], tp[:])

        # ---- c_ww = wg*c_wr_tm1 + (1-wg) + c_wlu_tm1 (full width) ----
        cww_all = big.tile([128, NT, B], F32)
        ve.tensor_scalar(cww_all[:], cwr_tm1[:], wg_b[:], onemwg_b[:], Alu.mult, Alu.add)
        ve.tensor_tensor(cww_all[:], cww_all[:], cwlu_tm1[:], Alu.add)

        cwr_all = big.tile([128, NT, B], F32R)
        cwu_all = big.tile([128, NT, B], F32)
        cwlu_all = big.tile([128, NT, B], F32)
        cos_all = big.tile([128, NT, B], F32)

        # ---- 1/||m_row|| for every local row ----
        rmn_all = sm1.tile([128, NT], F32)
        for t in range(NT):
            scr2 = sc2.tile([128, U], F32, name="scr2", bufs=1)
            se.activation(scr2[:], f(m_s[:, t, :]), Act.Square,
                          accum_out=rmn_all[:, t:t + 1])
        ve.tensor_scalar_max(rmn_all[:], rmn_all[:], EPS)
        msq_all = sm1.tile([128, NT], F32)
        se.activation(msq_all[:], rmn_all[:], Act.Sqrt)
        ve.reciprocal(rmn_all[:], msq_all[:])

        with (
            tc.tile_pool(name="cos_ps", bufs=2, space="PSUM") as cosp,
            tc.tile_pool(name="rd_ps", bufs=1, space="PSUM") as rdp,
            tc.tile_pool(name="wr_ps", bufs=2, space="PSUM") as wrp,
        ):
            read_ps = rdp.tile([B, U], F32)

            # ---- cos^T = n_key^T-contraction matmuls, 512-row groups ----
            for g in range(NG):
                mtg = mtgp.tile([128, UT, 512], F32R, name="mtg")
                for ti in range(4):
                    t = g * 4 + ti
                    for u in range(UT):
                        tp = trp.tile([128, 128], F32, name="tp")
                        te.transpose(r(tp[:]), m_s[:, t, u * 128:(u + 1) * 128],
                                     id128r[:])
                        ve.tensor_copy(mtg[:, u, ti * 128:(ti + 1) * 128], tp[:])
                cosT_ps = cosp.tile([B, 512], F32, name="cosT_ps")
                for u in range(UT):
                    te.matmul(cosT_ps[:], keyT[:, u, :], mtg[:, u, :],
                              start=(u == 0), stop=(u == UT - 1))
                cosT_sb = sc2.tile([B, 512], F32, name="cosT_sb", bufs=1)
                ve.tensor_copy(cosT_sb[:], cosT_ps[:])
                for ti in range(4):
                    t = g * 4 + ti
                    tpb = trp.tile([128, B], F32, name="tp")
                    te.transpose(tpb[:], cosT_sb[:, ti * 128:(ti + 1) * 128],
                                 id128[:B, :B])
                    ve.tensor_copy(cos_all[:, t, :], tpb[:])

            # ---- softmax over batch (full width, row-broadcast APs) ----
            def bcast(ap2d):  # (128, NT) -> (128, NT, B) step-0 broadcast
                return ap2d.unsqueeze(2).broadcast_to((128, NT, B))

            ve.tensor_tensor(cos_all[:], cos_all[:], bcast(rmn_all[:]), Alu.mult)
            se.activation(cos_all[:], cos_all[:], Act.Exp)
            rsum = sm1.tile([128, NT], F32)
            ve.tensor_reduce(rsum[:], cos_all[:], X, Alu.add)
            rrec = sm1.tile([128, NT], F32)
            ve.reciprocal(rrec[:], rsum[:])
            ve.tensor_tensor(cwr_all[:], cos_all[:], bcast(rrec[:]), Alu.mult)

            # ---- usage update (full width); cwlu_all doubles as scratch ----
            ve.tensor_tensor(cwlu_all[:], f(cwr_all[:]), cww_all[:], Alu.add)
            ve.tensor_scalar(cwu_all[:], cwu_tm1[:], DECAY, None, Alu.mult)
            ve.tensor_tensor(cwu_all[:], cwu_all[:], cwlu_all[:], Alu.add)

            dma2(h["cwr_out"].ap().rearrange("(t p) b -> p t b", p=128),
                 f(cwr_all[:]))
            dma2(h["cwu_out"].ap().rearrange("(t p) b -> p t b", p=128),
                 cwu_all[:])

            # local column-min over the 2048 local rows
            minacc = sm1.tile([128, B], F32)
            ve.tensor_reduce(minacc[:], cwu_all[:].rearrange("p t b -> p b t"), X,
                             Alu.min)
            tpm = trp.tile([B, 128], F32, name="tp")
            te.transpose(tpm[:], minacc[:], id128[:])
            mint = sm1.tile([B, 128], F32)
            ve.tensor_copy(mint[:], tpm[:])
            colmin = sm1.tile([B, 1], F32)
            ve.tensor_reduce(colmin[:], mint[:], X, Alu.min)

            # ---- collective 1: tiny column-min AllGather (gates the tail) ----
            ccm_in = dram.tile([B, 1], F32)
            ccm_out = dram.tile([N_CORES * B, 1], F32, addr_space="Shared")
            dma(ccm_in[:], colmin[:])
            gp.collective_compute(
                "AllGather",
                Alu.bypass,
                ins=[ccm_in[:].opt()],
                outs=[ccm_out[:].opt()],
                replica_groups=[list(range(N_CORES))],
            )
            gathm = sm1.tile([B, N_CORES], F32)
            dma(gathm[:], ccm_out[:].rearrange("(c b) f -> b (c f)", b=B))
            gmin = sm1.tile([B, 1], F32)
            ve.tensor_reduce(gmin[:], gathm[:], X, Alu.min)

            # ---- partial read, accumulated over local row tiles ----
            for t in range(NT):
                te.matmul(read_ps[:], cwr_all[:, t, :], m_s[:, t, :],
                          start=(t == 0), stop=(t == NT - 1))

            # ---- write term s = m + c_ww @ h (independent of the collective) ----
            s_all = big.tile([128, NT, U], F32)
            for t in range(NT):
                tpw = trp.tile([B, 128], F32, name="tp")
                te.transpose(tpw[:], cww_all[:, t, :], id128[:])
                cwwT = sc2.tile([B, 128], F32R, name="cwwT")
                ve.tensor_copy(cwwT[:], tpw[:])
                wr_ps = wrp.tile([128, U], F32, name="wr_ps")
                te.matmul(wr_ps[:], cwwT[:], h_new[:], start=True, stop=True)
                ve.tensor_tensor(s_all[:, t, :], f(m_s[:, t, :]), wr_ps[:], Alu.add)

            # ---- collective 2: read partials (off the critical path) ----
            cc_in = sm1.tile([B, U], F32)
            ve.tensor_copy(cc_in[:], read_ps[:])
            ccb_in = dram.tile([B, U], F32)
            ccb_out = dram.tile([N_CORES * B, U], F32, addr_space="Shared")
            dma(ccb_in[:], cc_in[:])
            gp.collective_compute(
                "AllGather",
                Alu.bypass,
                ins=[ccb_in[:].opt()],
                outs=[ccb_out[:].opt()],
                replica_groups=[list(range(N_CORES))],
            )
            gath = sm1.tile([B, N_CORES, U], F32)
            dma2(gath[:], ccb_out[:].rearrange("(c b) f -> b c f", b=B))

            read_full = sm1.tile([B, U], F32)
            ve.tensor_tensor(read_full[:], gath[:, 0, :], gath[:, 1, :], Alu.add)
            for c in range(2, N_CORES):
                ve.tensor_tensor(read_full[:], read_full[:], gath[:, c, :], Alu.add)
            dma2(h["read_out"].ap(), read_full[:])

            tpg = trp.tile([1, B], F32, name="tp")
            te.transpose(tpg[:], gmin[:], id128[:B, :B])
            gminrow = sm1.tile([1, B], F32)
            ve.tensor_copy(gminrow[:], tpg[:])
            gmin_b = sm1.tile([128, B], F32)
            gp.partition_broadcast(gmin_b[:], gminrow[:])

            # ---- least-used mask (full width) ----
            gmin_bc = gmin_b[:].unsqueeze(1).broadcast_to((128, NT, B))
            ve.tensor_tensor(cwlu_all[:], cwu_all[:], gmin_bc, Alu.is_le)
            rowflag = sm1.tile([128, NT], F32)
            ve.tensor_reduce(rowflag[:], cwlu_all[:], X, Alu.max)

            # ---- memory write: m_new = s - m*rowflag ----
            m_out_view = h["m_out"].ap().rearrange("(t p) u -> p t u", p=128)
            for t in range(NT):
                corr = sc2.tile([128, U], F32, name="scr2", tag="scr2", bufs=1)
                if t % 2 == 0:
                    se.activation(corr[:], f(m_s[:, t, :]), Act.Copy,
                                  scale=rowflag[:, t:t + 1])
                else:
                    ve.tensor_scalar_mul(corr[:], f(m_s[:, t, :]),
                                         rowflag[:, t:t + 1])
                ve.tensor_tensor(s_all[:, t, :], s_all[:, t, :], corr[:],
                                 Alu.subtract)
                (dma if t % 2 == 0 else dma2)(m_out_view[:, t, :], s_all[:, t, :])

        # ---- remaining outputs ----
        dma(h["cwlu_out"].ap().rearrange("(t p) b -> p t b", p=128), cwlu_all[:])


def _build():
    nc = bacc.Bacc("TRN2", target_bir_lowering=False, debug=False,
                   num_devices=N_CORES)
    h = {}
    h["x_in"] = nc.dram_tensor("x_in", [B, IN_DIM], F32, kind="ExternalInput")
    h["h_in"] = nc.dram_tensor("h_in", [B, U], F32, kind="ExternalInput")
    h["c_in"] = nc.dram_tensor("c_in", [B, U], F32, kind="ExternalInput")
    h["r_in"] = nc.dram_tensor("r_in", [B, U], F32, kind="ExternalInput")
    h["m_in"] = nc.dram_tensor("m_in", [MS, U], F32, kind="ExternalInput")
    h["cwu_in"] = nc.dram_tensor("cwu_in", [MS, B], F32, kind="ExternalInput")
    h["cwlu_in"] = nc.dram_tensor("cwlu_in", [MS, B], F32, kind="ExternalInput")
    h["cwr_in"] = nc.dram_tensor("cwr_in", [MS, B], F32, kind="ExternalInput")
    h["wk"] = nc.dram_tensor("wk", [IN_DIM, 4 * U], F32, kind="ExternalInput")
    h["wr"] = nc.dram_tensor("wr", [U, 5 * U], F32, kind="ExternalInput")
    h["bias"] = nc.dram_tensor("bias", [1, 4 * U], F32, kind="ExternalInput")
    h["wg"] = nc.dram_tensor("wg", [1, 1], F32, kind="ExternalInput")

    h["h_out"] = nc.dram_tensor("h_out", [B, U], F32, kind="ExternalOutput")
    h["c_out"] = nc.dram_tensor("c_out", [B, U], F32, kind="ExternalOutput")
    h["read_out"] = nc.dram_tensor("read_out", [B, U], F32, kind="ExternalOutput")
    h["m_out"] = nc.dram_tensor("m_out", [MS, U], F32, kind="ExternalOutput")
    h["cwu_out"] = nc.dram_tensor("cwu_out", [MS, B], F32, kind="ExternalOutput")
    h["cwlu_out"] = nc.dram_tensor("cwlu_out", [MS, B], F32, kind="ExternalOutput")
    h["cwr_out"] = nc.dram_tensor("cwr_out", [MS, B], F32, kind="ExternalOutput")

    with tile.TileContext(nc) as tc:
        _emit(nc, tc, h)
    nc.compile()
    return nc


def _get_nc():
    if "nc" not in _cache:
        _cache["nc"] = _build()
    return _cache["nc"]


def _run(inputs, trace=False):
    nc = _get_nc()
    f32 = np.float32

    def arr(x):
        return np.ascontiguousarray(np.asarray(x, dtype=f32))

    full = {k: arr(v) for k, v in inputs.items()}
    in_maps = []
    for k in range(N_CORES):
        sl = slice(k * MS, (k + 1) * MS)
        in_maps.append({
            "x_in": full["inputs"],
            "h_in": full["h_tm1"],
            "c_in": full["c_tm1"],
            "r_in": full["r_tm1"],
            "m_in": full["m_tm1"][sl],
            "cwu_in": full["c_wu_tm1"][sl],
            "cwlu_in": full["c_wlu_tm1"][sl],
            "cwr_in": full["c_wr_tm1"][sl],
            "wk": full["kernel"],
            "wr": full["recurrent_kernel"],
            "bias": full["bias"].reshape(1, 4 * U),
            "wg": full["write_gate"].reshape(1, 1),
        })
    br = run_bass_kernel_spmd(nc, in_maps, core_ids=list(range(N_CORES)),
                              trace=trace)
    res = br.results
    h_out = res[0]["h_out"]
    c_out = res[0]["c_out"]
    read_out = res[0]["read_out"]
    m_out = np.concatenate([res[k]["m_out"] for k in range(N_CORES)], axis=0)
    cwu_out = np.concatenate([res[k]["cwu_out"] for k in range(N_CORES)], axis=0)
    cwlu_out = np.concatenate([res[k]["cwlu_out"] for k in range(N_CORES)], axis=0)
    cwr_out = np.concatenate([res[k]["cwr_out"] for k in range(N_CORES)], axis=0)
    out = (h_out, c_out, read_out, m_out, cwu_out, cwlu_out, cwr_out)
    return out, br


def kernel(**inputs):
    out, _ = _run(inputs, trace=False)
    return out
